# revision 24
# baseline (speedup 1.0000x reference)
"""Trainium2 Bass kernel: 4-layer decode-attention transformer block (fp8 KV).

Shapes (hardcoded): L=4, B=32, H=8, Dh=64, D=512, TP=1024, TN=3, Tt=1027.
Sharding: data-parallel over B across 8 cores (4 envs each); params replicated.

v5 design notes (evolved from the 234.6us bf16 v2 baseline; v4 fp8 measured
210.9us, PE-bound on instruction count):
 - K/V/Wo streamed from HBM in fp8 e4m3 (~20.5MB/core vs 38.8MB).
 - E packed [128, 512] per env: row = 32*(2g + h2) + m, col = key t within
   the h2 half (m = 6i + 3hf + tq labels the two pair-blocks g+2i). This
   packing lets ONE Act exp call (512 cols) cover the whole env, with
   accum_out producing the softmax denominators for free.
 - QK^T is 8 plain fp8 matmuls [32,512] per env (DoubleRow outputs must
   start at partition 0 - walrus codegen constraint - so DR can't write the
   row-packed E); AV, denominator-fold tails, new-token E, and the Wo
   projection DO use fp8 DoubleRow with base-0 outputs.
 - A^T via 4 PE transposes [128,128] -> one scalar.copy; the AT column
   layout 128*jj + 64g + 32i + m falls out with i = h2 as the DoubleRow
   k-tile pair (t, t+512), matching the vF host layout.
 - Denominator: exp accum_out [128,1] folded pairwise by 2 Act Identity ops
   with cross-partition-base bias (engines allow that only via Act bias);
   plus tiny new-token ones-matmuls; rcp on DVE.
 - O gather: plain matmuls with identity slices select the 6 valid rows per
   (g,b) out of onrm^T, so the CT gather is 2 strided copies (on Pool).
 - Pool engine (nc.gpsimd) offloads residual adds, copies, quake rsqrt.
 - LN: DVE bn_stats/bn_aggr; 1/sqrt(var) via quake bit-trick + 2 Newton
   iterations on Pool (keeps Act tables pinned to the exp set).
"""

import numpy as np

L, B, H, Dh, D, TP, TN = 4, 32, 8, 64, 512, 1024, 3
Tt = TP + TN
NC = 8
BB = B // NC          # envs per core = 4
R = BB * TN           # x rows per core = 12
NJ = TP // 128        # t-chunks of 128 = 8
NPAIR = H // 2        # head pairs = 4
EPS = 1e-5
NEG = -1e9
ESC = 0.125           # 1/sqrt(Dh), applied as Act scale at exp time
QMAGIC = 1597463007.0  # 0x5f3759df


def _build_bass(fast=True):
    import concourse.bass as bass
    import concourse.mybir as mybir
    import concourse.tile as tile
    from concourse import bacc

    f32 = mybir.dt.float32
    i32 = mybir.dt.int32
    bf16 = mybir.dt.bfloat16
    f8 = mybir.dt.float8e4
    AF = mybir.ActivationFunctionType
    OP = mybir.AluOpType
    DR = mybir.MatmulPerfMode.DoubleRow

    nc = bacc.Bacc("TRN2", target_bir_lowering=False, debug=False, num_devices=NC)

    x_d = nc.dram_tensor("x0", [R, D], f32, kind="ExternalInput")
    # K^T per (l, env): rows 64*hf+d, cols 1024*pr + t
    kt_d = nc.dram_tensor("ktT", [L, BB, 128, NPAIR * TP], f8, kind="ExternalInput")
    # V per (l, env): rows p, cols 1024*jp + 512*g + 256*i + 128*b + 64*hf + d
    #  = V[head 4b+2g+hf, t = 512*i + 128*jp + p, d]
    vf_d = nc.dram_tensor("vF", [L, BB, 128, NPAIR * TP], f8, kind="ExternalInput")
    wq_d = nc.dram_tensor("wq2", [L, 128, 128], bf16, kind="ExternalInput")
    wk_d = nc.dram_tensor("wk2", [L, 128, 128], bf16, kind="ExternalInput")
    wv_d = nc.dram_tensor("wv2", [L, 128, 128], bf16, kind="ExternalInput")
    # Wo as fp8 DR pairs: col = 1024g + 512b + n  (c = g + 2b)
    wo_d = nc.dram_tensor("wo2", [L, 128, 2 * D * 2], f8, kind="ExternalInput")
    wf_d = nc.dram_tensor("wfT", [L, 4, 128, D], bf16, kind="ExternalInput")
    p12_d = nc.dram_tensor("p12", [L, 6, R, D], f32, kind="ExternalInput")
    i128_d = nc.dram_tensor("i128b", [128, 128], bf16, kind="ExternalInput")
    ones_d = nc.dram_tensor("ones2f8", [128, 32], f8, kind="ExternalInput")
    # new-token causal/pad bias, rows tn, cols 64e + 32g + m
    negnT_d = nc.dram_tensor("negnT", [16, 256], f32, kind="ExternalInput")
    # padded-slot count, rows m, col 2e + g
    npad_d = nc.dram_tensor("npad4", [32, 2 * BB], f32, kind="ExternalInput")
    out_d = nc.dram_tensor("xout", [R, D], f32, kind="ExternalOutput")

    from contextlib import ExitStack

    with tile.TileContext(nc) as tc, ExitStack() as st:
        consts = st.enter_context(tc.tile_pool(name="consts", bufs=1))
        sb = st.enter_context(tc.tile_pool(name="sb", bufs=1))
        ps = st.enter_context(tc.tile_pool(name="ps", bufs=1, space="PSUM"))

        x = consts.tile([R, D], f32)
        nc.sync.dma_start(x[:], x_d[:])
        i128 = consts.tile([128, 128], bf16)
        nc.sync.dma_start(i128[:], i128_d[:])
        o2f8 = consts.tile([128, 32], f8)
        nc.sync.dma_start(o2f8[:], ones_d[:])
        negnT = consts.tile([16, 256], f32)
        nc.sync.dma_start(negnT[:], negnT_d[:])
        npad4 = consts.tile([32, 2 * BB], f32)
        nc.sync.dma_start(npad4[:], npad_d[:])
        qmag = consts.tile([R, 1], i32)
        nc.vector.memset(qmag[:], QMAGIC)

        wq2all = consts.tile([128, L * 128], bf16)
        nc.sync.dma_start(wq2all.rearrange("p (l n) -> p l n", l=L),
                          wq_d.rearrange("l p n -> p l n"))
        wk2all = consts.tile([128, L * 128], bf16)
        nc.sync.dma_start(wk2all.rearrange("p (l n) -> p l n", l=L),
                          wk_d.rearrange("l p n -> p l n"))
        wv2all = consts.tile([128, L * 128], bf16)
        nc.sync.dma_start(wv2all.rearrange("p (l n) -> p l n", l=L),
                          wv_d.rearrange("l p n -> p l n"))

        for l in range(L):
            # ---- per-layer loads ----
            ktA = sb.tile([128, BB * NPAIR * TP], f8, tag="ktA", bufs=2,
                          name=f"ktA_{l}")
            nc.sync.dma_start(ktA.rearrange("p (e n) -> p e n", e=BB),
                              kt_d[l].rearrange("e p n -> p e n"))
            vFA = sb.tile([128, BB * NPAIR * TP], f8, tag="vFA", bufs=2,
                          name=f"vFA_{l}")
            nc.sync.dma_start(vFA.rearrange("p (e n) -> p e n", e=BB),
                              vf_d[l].rearrange("e p n -> p e n"))
            wo_t = sb.tile([128, 4 * D], f8, tag="wo", bufs=2, name=f"wo_{l}")
            nc.sync.dma_start(wo_t[:], wo_d[l])
            wf_t = sb.tile([128, 4 * D], bf16, tag="wf", bufs=2, name=f"wf_{l}")
            nc.sync.dma_start(wf_t.rearrange("p (c n) -> p c n", c=4),
                              wf_d[l].rearrange("c p n -> p c n"))
            if not fast:
                p12_t = sb.tile([R, 6 * D], f32, tag="p12", bufs=1,
                                name=f"p12_{l}")
                nc.sync.dma_start(p12_t.rearrange("p (g n) -> p g n", g=6),
                                  p12_d[l].rearrange("g p n -> p g n"))
                ln1w = p12_t[:, 0 * D: 1 * D]
                ln1b = p12_t[:, 1 * D: 2 * D]
                ln2w = p12_t[:, 2 * D: 3 * D]
                ln2b = p12_t[:, 3 * D: 4 * D]
                bo12 = p12_t[:, 4 * D: 5 * D]
                bf12 = p12_t[:, 5 * D: 6 * D]
            else:
                ln1w = ln1b = ln2w = ln2b = bo12 = bf12 = None

            def layer_norm(xin, wln, bln):
                st6 = sb.tile([R, 6], f32, tag="lnst", bufs=2)
                nc.vector.bn_stats(st6[:], xin)
                mv = sb.tile([R, 2], f32, tag="lnmv", bufs=2)
                nc.vector.bn_aggr(mv[:], st6[:])
                # rs = 1/sqrt(var) via quake bit-trick + 2 Newton iters on
                # Pool (avoids Act Sqrt/Ln so the exp table never reloads;
                # eps=1e-5 is negligible vs var~1)
                yq = sb.tile([R, 1], f32, tag="lnyq", bufs=2)
                yi = yq.bitcast(i32)
                nc.vector.tensor_scalar(yi, mv[:, 1:2].bitcast(i32), 1, 0,
                                        OP.arith_shift_right, OP.bitwise_xor)
                nc.vector.tensor_tensor(out=yi, in0=qmag[:], in1=yi,
                                        op=OP.subtract)
                sq = sb.tile([R, 1], f32, tag="lnsq", bufs=2)
                for _ in range(2):
                    nc.gpsimd.tensor_tensor(out=sq[:], in0=yq[:], in1=yq[:],
                                            op=OP.mult)
                    nc.gpsimd.tensor_tensor(out=sq[:], in0=sq[:],
                                            in1=mv[:, 1:2], op=OP.mult)
                    nc.gpsimd.tensor_scalar(sq[:], sq[:], -0.5, 1.5,
                                            OP.mult, OP.add)
                    nc.gpsimd.tensor_tensor(out=yq[:], in0=yq[:], in1=sq[:],
                                            op=OP.mult)
                hb = sb.tile([R, D], bf16, tag="lnhb", bufs=2)
                if fast:
                    nc.gpsimd.tensor_scalar(hb[:], xin, mv[:, 0:1], yq[:],
                                            OP.subtract, OP.mult)
                    return hb
                hh = sb.tile([R, D], f32, tag="lnh", bufs=2)
                nc.vector.tensor_scalar(hh[:], xin, mv[:, 0:1], yq[:],
                                        OP.subtract, OP.mult)
                nc.vector.tensor_tensor(out=hh[:], in0=hh[:], in1=wln, op=OP.mult)
                nc.vector.tensor_tensor(out=hb[:], in0=hh[:], in1=bln, op=OP.add)
                return hb

            h1 = layer_norm(x[:], ln1w, ln1b)

            # ---- hT2all [128, 48] bf16: h^T, col = 12c + 3e + tq ----
            hT2all = sb.tile([128, 4 * R], bf16, tag="hT2", bufs=2,
                             name=f"hT2_{l}")
            for c in range(NPAIR):
                tp = ps.tile([128, R], bf16, tag="sm", bufs=3)
                nc.tensor.transpose(tp[:], h1[:, 128 * c: 128 * (c + 1)],
                                    i128[0:R, 0:R])
                nc.vector.tensor_copy(hT2all[:, 12 * c: 12 * (c + 1)], tp[:])

            # ---- QKV projections (one matmul each for Q^T, K^T) ----
            q2d = sb.tile([128, 512], f8, tag="q2d", bufs=2, name=f"q2d_{l}")
            nc.gpsimd.memset(q2d[:], 0.0)
            q2dv = q2d.rearrange("p (g e i m) -> p g e i m", g=2, e=BB, i=2)
            knT = sb.tile([128, 256], f8, tag="knT", bufs=2, name=f"knT_{l}")
            nc.gpsimd.memset(knT[:], 0.0)
            knv = knT.rearrange("p (e g i m) -> p e g i m", e=BB, g=2, i=2)

            qT2all = ps.tile([128, 4 * R], f32, tag="sm", bufs=3)
            nc.tensor.matmul(qT2all[:], wq2all[:, 128 * l: 128 * (l + 1)],
                             hT2all[:])
            qv = qT2all.rearrange("p (c e t) -> p c e t", c=4, e=BB)
            for c in range(NPAIR):
                g, i = c % 2, c // 2
                for hf in range(2):
                    nc.vector.tensor_copy(
                        q2dv[64 * hf: 64 * hf + 64, g, :, i,
                             6 * i + 3 * hf: 6 * i + 3 * hf + 3],
                        qv[64 * hf: 64 * hf + 64, c, :, :])
            kT2all = ps.tile([128, 4 * R], f32, tag="sm", bufs=3)
            nc.tensor.matmul(kT2all[:], wk2all[:, 128 * l: 128 * (l + 1)],
                             hT2all[:])
            kv = kT2all.rearrange("p (i g e t) -> p e g i t", i=2, g=2, e=BB)
            for i in range(2):
                nc.vector.tensor_copy(knv[:, :, :, i, 0:TN], kv[:, :, :, i, :])
            vn_ps = ps.tile([R, D], f32, tag="vn", bufs=1)
            for c in range(NPAIR):
                nc.tensor.matmul(vn_ps[:, 128 * c: 128 * (c + 1)],
                                 hT2all[:, 12 * c: 12 * (c + 1)],
                                 wv2all[:, 128 * l: 128 * (l + 1)])
            vnsb = sb.tile([R, D], f8, tag="vnsb", bufs=2)
            nc.scalar.copy(vnsb[:], vn_ps[:])
            Vn = [sb.tile([TN, D], f8, tag="Vn", bufs=8, name=f"Vn_{l}_{e}")
                  for e in range(BB)]
            for e in range(BB):
                nc.sync.dma_start(Vn[e][:], vnsb[3 * e: 3 * e + TN, :])

            # ---- new-token E, transposed: EnT[tn, 64e + 32g + m] ----
            EnT = ps.tile([16, 256], f32, tag="sm", bufs=3)
            nc.vector.memset(EnT[:], 0.0)
            for e in range(BB):
                for g in range(2):
                    nc.tensor.matmul(
                        EnT[0:16, 64 * e + 32 * g: 64 * e + 32 * g + 32],
                        knv[:, e, g, :, :], q2dv[:, g, e, :, :],
                        perf_mode=DR)
            nc.vector.tensor_tensor(out=EnT[:], in0=EnT[:], in1=negnT[:],
                                    op=OP.add)
            AnT = sb.tile([16, 256], f8, tag="AnT", bufs=2, name=f"AnT_{l}")
            nc.scalar.activation(AnT[:], EnT[:], AF.Exp, scale=ESC)

            # ---- attention per env, software-pipelined ----
            CTall = sb.tile([128, 64], f8, tag="CT", bufs=2, name=f"CT_{l}")
            nc.gpsimd.memset(CTall[:], 0.0)
            ctv = CTall.rearrange("p (g b m) -> p g b m", g=2, b=2)
            vFv = vFA.rearrange("p (e j g i b dd) -> p e j g i b dd",
                                e=BB, j=4, g=2, i=2, b=2)

            def emit_E(e):
                E = ps.tile([128, 512], f32, tag="eb", bufs=2,
                            name=f"E_{l}_{e}")
                for g in range(2):
                    for h2 in range(2):
                        rb = 32 * (2 * g + h2)
                        for i in range(2):
                            nc.tensor.matmul(
                                E[rb: rb + 32, 0:512],
                                q2dv[:, g, e, i, :],
                                ktA[:, 4096 * e + 1024 * (g + 2 * i) + 512 * h2:
                                    4096 * e + 1024 * (g + 2 * i) + 512 * (h2 + 1)],
                                start=(i == 0), stop=(i == 1),
                                tile_position=(0, rb))
                A4 = sb.tile([128, 512], bf16, tag="A4", bufs=2,
                             name=f"A4_{l}_{e}")
                acc = sb.tile([128, 1], f32, tag="acc", bufs=2)
                nc.scalar.activation(A4[:], E[:], AF.Exp, scale=ESC,
                                     accum_out=acc[:])
                # A^T: col = 128jj + 64g + 32i + m  (i = h2 = t-half)
                ATp = ps.tile([128, 512], bf16, tag="sm", bufs=3)
                for jj in range(4):
                    nc.tensor.transpose(ATp[:, 128 * jj: 128 * (jj + 1)],
                                        A4[:, 128 * jj: 128 * (jj + 1)],
                                        i128[:, :])
                AT = sb.tile([128, 512], f8, tag="AT", bufs=2,
                             name=f"AT_{l}_{e}")
                nc.scalar.copy(AT[:], ATp[:])
                ATv = AT.rearrange("p (j g i m) -> p j g i m", j=4, g=2, i=2)
                # denominators: fold h2 halves of acc (Act bias allows the
                # cross-partition-base read), add new-token sums, rcp on DVE
                dnew = ps.tile([32, 4], f32, tag="sm", bufs=3)
                for g in range(2):
                    nc.tensor.matmul(
                        dnew[0:32, 2 * g: 2 * g + 2],
                        AnT[0:TN, 64 * e + 32 * g: 64 * e + 32 * g + 32],
                        o2f8[0:TN, 0:2])
                ds = sb.tile([32, 2], f32, tag="ds", bufs=2)
                for g in range(2):
                    nc.scalar.activation(ds[:, g: g + 1],
                                         acc[64 * g: 64 * g + 32, :],
                                         AF.Identity,
                                         bias=acc[64 * g + 32: 64 * g + 64, :])
                rcp = sb.tile([32, 2], f32, tag="rcp", bufs=2)
                nc.vector.tensor_tensor(
                    out=rcp[:], in0=ds[:],
                    in1=dnew.rearrange("p (g o) -> p g o", g=2)[:, :, 0],
                    op=OP.add)
                nc.vector.tensor_tensor(out=rcp[:], in0=rcp[:],
                                        in1=npad4[:, 2 * e: 2 * e + 2],
                                        op=OP.subtract)
                nc.vector.reciprocal(rcp[:], rcp[:])
                return ATv, rcp

            def emit_attn(e, ATv, rcp):
                otp = ps.tile([128, 48], f32, tag="sm", bufs=3)
                for g in range(2):
                    og = ps.tile([32, 256], f32, tag="og", bufs=2,
                                 name=f"og_{l}_{e}_{g}")
                    for jp in range(4):
                        nc.tensor.matmul(
                            og[0:32, 0:256],
                            ATv[:, jp, g, :, :],
                            vFv[:, e, jp, g, :, :, :],
                            perf_mode=DR, start=(jp == 0), stop=False)
                    nc.tensor.matmul(
                        og[0:32, 0:256],
                        AnT[0:TN, 64 * e + 32 * g: 64 * e + 32 * g + 32],
                        Vn[e].rearrange("p (b d2) -> p b d2", b=2)[
                            :, :, 128 * g: 128 * (g + 1)],
                        start=False, stop=True)
                    onrm = sb.tile([32, 256], bf16, tag="onrm", bufs=4)
                    nc.vector.tensor_scalar_mul(onrm[:], og[:],
                                                rcp[:, g: g + 1])
                    # O^T gather: identity-slice matmuls select the 6 valid
                    # rows per (g, b); otp col = 12c + 3hf + tq
                    for b in range(2):
                        c = g + 2 * b
                        nc.tensor.matmul(
                            otp[:, 12 * c: 12 * c + 6],
                            onrm[:, 128 * b: 128 * (b + 1)],
                            i128[0:32, 6 * b: 6 * b + 6])
                src = otp.rearrange("p (b g u) -> p g b u", b=2, g=2)
                for hf in range(2):
                    nc.vector.tensor_copy(
                        ctv[64 * hf: 64 * hf + 64, :, :, 3 * e: 3 * e + 3],
                        src[64 * hf: 64 * hf + 64, :, :,
                            3 * hf: 3 * hf + 3])

            pend = emit_E(0)
            for e in range(BB):
                nxt = emit_E(e + 1) if e + 1 < BB else None
                emit_attn(e, *pend)
                pend = nxt

            # ---- output projection (fp8 DR over b-pairs) + residual ----
            xo = ps.tile([16, D], f32, tag="sm", bufs=3)
            wov = wo_t.rearrange("p (g b n) -> p g b n", g=2, b=2)
            for nh in range(2):
                for g in range(2):
                    nc.tensor.matmul(
                        xo[0:16, 256 * nh: 256 * (nh + 1)],
                        ctv[:, g, :, :],
                        wov[:, g, :, 256 * nh: 256 * (nh + 1)],
                        perf_mode=DR, start=(g == 0), stop=(g == 1))
            if fast:
                nc.vector.tensor_tensor(out=x[:], in0=x[:], in1=xo[0:R, :],
                                        op=OP.add)
            else:
                xt = sb.tile([R, D], f32, tag="scr", bufs=2)
                nc.vector.tensor_tensor(out=xt[:], in0=xo[0:R, :], in1=bo12,
                                        op=OP.add)
                nc.vector.tensor_tensor(out=x[:], in0=x[:], in1=xt[:], op=OP.add)

            # ---- FFN ----
            h2t = layer_norm(x[:], ln2w, ln2b)
            HTall = sb.tile([128, 4 * R], bf16, tag="HT", bufs=2,
                            name=f"HT_{l}")
            for c in range(NPAIR):
                tp = ps.tile([128, R], bf16, tag="sm", bufs=3)
                nc.tensor.transpose(tp[:], h2t[:, 128 * c: 128 * (c + 1)],
                                    i128[0:R, 0:R])
                nc.vector.tensor_copy(HTall[:, 12 * c: 12 * (c + 1)], tp[:])
            ff = ps.tile([R, D], f32, tag="sm", bufs=3)
            for c in range(NPAIR):
                nc.tensor.matmul(ff[:], HTall[:, 12 * c: 12 * (c + 1)],
                                 wf_t[:, c * D: (c + 1) * D],
                                 start=(c == 0), stop=(c == NPAIR - 1))
            ft = sb.tile([R, D], f32, tag="scr", bufs=2)
            if fast:
                nc.scalar.activation(ft[:], ff[:], AF.Relu)
            else:
                nc.vector.tensor_tensor(out=ft[:], in0=ff[:], in1=bf12, op=OP.add)
                nc.scalar.activation(ft[:], ft[:], AF.Relu)
            nc.vector.tensor_tensor(out=x[:], in0=x[:], in1=ft[:], op=OP.add)

        nc.sync.dma_start(out_d[:], x[:])

    nc.compile()
    return nc


def _prep_inputs(x, past_k, past_v, pad_mask, ln1_w, ln1_b, ln2_w, ln2_b,
                 Wq, Wk, Wv, Wo, bo, Wf, bf):
    import ml_dtypes
    f = np.float32
    b16 = ml_dtypes.bfloat16
    fp8 = ml_dtypes.float8_e4m3
    x = np.ascontiguousarray(x, f)
    past_k = np.asarray(past_k, f)
    past_v = np.asarray(past_v, f)
    pad_mask = np.asarray(pad_mask)

    def blk2(wT):
        out = np.zeros((L, 128, 128), f)
        out[:, 0:64, 0:64] = wT
        out[:, 64:128, 64:128] = wT
        return out.astype(b16)
    wq2 = blk2(np.transpose(np.asarray(Wq, f), (0, 2, 1)))
    wk2 = blk2(np.transpose(np.asarray(Wk, f), (0, 2, 1)))
    wv2 = blk2(np.transpose(np.asarray(Wv, f), (0, 2, 1)))
    woT = np.transpose(np.asarray(Wo, f), (0, 2, 1)).reshape(L, 2, 2, 128, D)
    # wo2[l, p, 1024g + 512b + n] = Wo^T[l, 128(g+2b) + p, n]; c = 2b + g in
    # woT's reshape is (b, g) so transpose to (l, p, g, b, n)
    wo2 = np.transpose(woT, (0, 3, 2, 1, 4)).reshape(L, 128, 2048)
    wo2 = np.clip(wo2, -240.0, 240.0).astype(fp8)
    wfT = np.transpose(np.asarray(Wf, f), (0, 2, 1)).reshape(L, 4, 128, D).astype(b16)
    p12 = np.stack(
        [np.broadcast_to(np.asarray(a, f)[:, None, :], (L, R, D))
         for a in (ln1_w, ln1_b, ln2_w, ln2_b, bo, bf)], axis=1)
    p12 = np.ascontiguousarray(p12)
    i128 = np.eye(128, dtype=b16)
    ones2 = np.ones((128, 32), f).astype(fp8)

    def to8(a):
        return np.clip(a, -240.0, 240.0).astype(fp8)

    in_maps = []
    for cc in range(NC):
        bs = slice(cc * BB, (cc + 1) * BB)
        pk = past_k[:, bs]                      # (L, BB, H, TP, Dh)
        pv = past_v[:, bs]
        # ktT[l, e, 64hf+d, 1024pr+t] = pk[l, e, 2pr+hf, t, d]
        kt = pk.reshape(L, BB, NPAIR, 2, TP, Dh)
        kt = np.transpose(kt, (0, 1, 3, 5, 2, 4))    # l, e, hf, d, pr, t
        kt = to8(np.ascontiguousarray(kt.reshape(L, BB, 128, NPAIR * TP)))
        # vF[l, e, p, 1024jp+512g+256i+128b+64hf+d] =
        #    pv[l, e, 4b+2g+hf, 512i+128jp+p, d]
        vf = pv.reshape(L, BB, 2, 2, 2, 2, NJ // 2, 128, Dh)
        #                      b  g  hf i  jp      p    d
        vf = np.transpose(vf, (0, 1, 7, 6, 3, 5, 2, 4, 8))
        #   -> l, e, p, jp, g, i, b, hf, d
        vf = to8(np.ascontiguousarray(vf.reshape(L, BB, 128, NPAIR * TP)))

        pm = np.asarray(pad_mask[bs])           # (BB, Tt) bool
        npad_e = (TP - pm[:, :TP].sum(axis=1)).astype(f)   # (BB,)

        npad4 = np.zeros((32, 2 * BB), f)
        for e in range(BB):
            npad4[0:12, 2 * e: 2 * e + 2] = npad_e[e]
        negnT = np.full((16, 256), NEG, f)
        for e in range(BB):
            for tn in range(TN):
                for g in range(2):
                    for m in range(12):
                        tq = m % 3
                        if tn <= tq and bool(pm[e, TP + tn]):
                            negnT[tn, 64 * e + 32 * g + m] = 0.0

        in_maps.append({
            "x0": np.ascontiguousarray(x[bs].reshape(R, D)),
            "ktT": kt, "vF": vf,
            "wq2": wq2, "wk2": wk2, "wv2": wv2,
            "wo2": wo2, "wfT": wfT, "p12": p12,
            "i128b": i128, "ones2f8": ones2,
            "negnT": negnT, "npad4": npad4,
        })
    return in_maps


_CACHE = {}


def kernel(**inputs):
    import os
    import sys
    for p in ("/opt/trn_rl_repo", "/opt/pypackages"):
        if p not in sys.path:
            sys.path.insert(0, p)
    os.environ.setdefault("JAX_PLATFORMS", "")
    from concourse.bass_utils import run_bass_kernel_spmd

    in_maps = _prep_inputs(**inputs)
    fast = all(np.allclose(np.asarray(inputs[k]), 1.0) for k in ("ln1_w", "ln2_w")) \
        and all(np.allclose(np.asarray(inputs[k]), 0.0)
                for k in ("ln1_b", "ln2_b", "bo", "bf"))
    key = f"nc_{fast}"
    if key not in _CACHE:
        _CACHE[key] = _build_bass(fast)
    nc = _CACHE[key]
    res = run_bass_kernel_spmd(nc, in_maps, core_ids=list(range(NC)))
    out = np.concatenate([r["xout"].reshape(BB, TN, D) for r in res.results], axis=0)
    return out.astype(np.float32)


# revision 25
# speedup vs baseline: 1.3931x; 1.3931x over previous
"""Trainium2 Bass kernel: 4-layer decode-attention transformer block (fp8 KV).

Shapes (hardcoded): L=4, B=32, H=8, Dh=64, D=512, TP=1024, TN=3, Tt=1027.
Sharding: data-parallel over B across 8 cores (4 envs each); params replicated.

v5 design notes (evolved from the 234.6us bf16 v2 baseline; v4 fp8 measured
210.9us, PE-bound on instruction count):
 - K/V/Wo streamed from HBM in fp8 e4m3 (~20.5MB/core vs 38.8MB).
 - E packed [128, 512] per env: row = 32*(2g + h2) + m, col = key t within
   the h2 half (m = 6i + 3hf + tq labels the two pair-blocks g+2i). This
   packing lets ONE Act exp call (512 cols) cover the whole env, with
   accum_out producing the softmax denominators for free.
 - QK^T is 8 plain fp8 matmuls [32,512] per env (DoubleRow outputs must
   start at partition 0 - walrus codegen constraint - so DR can't write the
   row-packed E); AV, denominator-fold tails, new-token E, and the Wo
   projection DO use fp8 DoubleRow with base-0 outputs.
 - A^T via 4 PE transposes [128,128] -> one scalar.copy; the AT column
   layout 128*jj + 64g + 32i + m falls out with i = h2 as the DoubleRow
   k-tile pair (t, t+512), matching the vF host layout.
 - Denominator: exp accum_out [128,1] folded pairwise by 2 Act Identity ops
   with cross-partition-base bias (engines allow that only via Act bias);
   plus tiny new-token ones-matmuls; rcp on DVE.
 - O gather: plain matmuls with identity slices select the 6 valid rows per
   (g,b) out of onrm^T, so the CT gather is 2 strided copies (on Pool).
 - Pool engine (nc.gpsimd) offloads residual adds, copies, quake rsqrt.
 - LN: DVE bn_stats/bn_aggr; 1/sqrt(var) via quake bit-trick + 2 Newton
   iterations on Pool (keeps Act tables pinned to the exp set).
"""

import numpy as np

L, B, H, Dh, D, TP, TN = 4, 32, 8, 64, 512, 1024, 3
Tt = TP + TN
NC = 8
BB = B // NC          # envs per core = 4
R = BB * TN           # x rows per core = 12
NJ = TP // 128        # t-chunks of 128 = 8
NPAIR = H // 2        # head pairs = 4
EPS = 1e-5
NEG = -1e9
ESC = 0.125           # 1/sqrt(Dh), applied as Act scale at exp time
QMAGIC = 1597463007.0  # 0x5f3759df


def _build_bass(fast=True):
    import concourse.bass as bass
    import concourse.mybir as mybir
    import concourse.tile as tile
    from concourse import bacc

    f32 = mybir.dt.float32
    i32 = mybir.dt.int32
    bf16 = mybir.dt.bfloat16
    f8 = mybir.dt.float8e4
    AF = mybir.ActivationFunctionType
    OP = mybir.AluOpType
    DR = mybir.MatmulPerfMode.DoubleRow

    nc = bacc.Bacc("TRN2", target_bir_lowering=False, debug=False, num_devices=NC)

    x_d = nc.dram_tensor("x0", [R, D], f32, kind="ExternalInput")
    # K^T per (l, env): rows 64*hf+d, cols 1024*pr + t
    kt_d = nc.dram_tensor("ktT", [L, BB, 128, NPAIR * TP], f8, kind="ExternalInput")
    # V per (l, env): rows p, cols 1024*jp + 512*g + 256*i + 128*b + 64*hf + d
    #  = V[head 4b+2g+hf, t = 512*i + 128*jp + p, d]
    vf_d = nc.dram_tensor("vF", [L, BB, 128, NPAIR * TP], f8, kind="ExternalInput")
    wq_d = nc.dram_tensor("wq2", [L, 128, 128], bf16, kind="ExternalInput")
    wk_d = nc.dram_tensor("wk2", [L, 128, 128], bf16, kind="ExternalInput")
    wv_d = nc.dram_tensor("wv2", [L, 128, 128], bf16, kind="ExternalInput")
    # Wo as fp8 DR pairs: col = 1024g + 512b + n  (c = g + 2b)
    wo_d = nc.dram_tensor("wo2", [L, 128, 2 * D * 2], f8, kind="ExternalInput")
    wf_d = nc.dram_tensor("wfT", [L, 4, 128, D], bf16, kind="ExternalInput")
    p12_d = nc.dram_tensor("p12", [L, 6, R, D], f32, kind="ExternalInput")
    i128_d = nc.dram_tensor("i128b", [128, 128], bf16, kind="ExternalInput")
    ones_d = nc.dram_tensor("ones2f8", [128, 32], f8, kind="ExternalInput")
    # new-token causal/pad bias, rows tn, cols 64e + 32g + m
    negnT_d = nc.dram_tensor("negnT", [16, 256], f32, kind="ExternalInput")
    # padded-slot count, rows m, col 2e + g
    npad_d = nc.dram_tensor("npad4", [32, 2 * BB], f32, kind="ExternalInput")
    out_d = nc.dram_tensor("xout", [R, D], f32, kind="ExternalOutput")

    from contextlib import ExitStack

    with tile.TileContext(nc) as tc, ExitStack() as st:
        consts = st.enter_context(tc.tile_pool(name="consts", bufs=1))
        sb = st.enter_context(tc.tile_pool(name="sb", bufs=1))
        ps = st.enter_context(tc.tile_pool(name="ps", bufs=1, space="PSUM"))

        x = consts.tile([R, D], f32)
        nc.sync.dma_start(x[:], x_d[:])
        i128 = consts.tile([128, 128], bf16)
        nc.sync.dma_start(i128[:], i128_d[:])
        o2f8 = consts.tile([128, 32], f8)
        nc.sync.dma_start(o2f8[:], ones_d[:])
        negnT = consts.tile([16, 256], f32)
        nc.sync.dma_start(negnT[:], negnT_d[:])
        npad4 = consts.tile([32, 2 * BB], f32)
        nc.sync.dma_start(npad4[:], npad_d[:])
        qmag = consts.tile([R, 1], i32)
        nc.vector.memset(qmag[:], QMAGIC)

        wq2all = consts.tile([128, L * 128], bf16)
        nc.sync.dma_start(wq2all.rearrange("p (l n) -> p l n", l=L),
                          wq_d.rearrange("l p n -> p l n"))
        wk2all = consts.tile([128, L * 128], bf16)
        nc.sync.dma_start(wk2all.rearrange("p (l n) -> p l n", l=L),
                          wk_d.rearrange("l p n -> p l n"))
        wv2all = consts.tile([128, L * 128], bf16)
        nc.sync.dma_start(wv2all.rearrange("p (l n) -> p l n", l=L),
                          wv_d.rearrange("l p n -> p l n"))

        for l in range(L):
            # ---- per-layer loads ----
            ktA = sb.tile([128, BB * NPAIR * TP], f8, tag="ktA", bufs=2,
                          name=f"ktA_{l}")
            nc.sync.dma_start(ktA.rearrange("p (e n) -> p e n", e=BB),
                              kt_d[l].rearrange("e p n -> p e n"))
            vFA = sb.tile([128, BB * NPAIR * TP], f8, tag="vFA", bufs=2,
                          name=f"vFA_{l}")
            nc.sync.dma_start(vFA.rearrange("p (e n) -> p e n", e=BB),
                              vf_d[l].rearrange("e p n -> p e n"))
            wo_t = sb.tile([128, 4 * D], f8, tag="wo", bufs=2, name=f"wo_{l}")
            nc.sync.dma_start(wo_t[:], wo_d[l])
            wf_t = sb.tile([128, 4 * D], bf16, tag="wf", bufs=2, name=f"wf_{l}")
            nc.sync.dma_start(wf_t.rearrange("p (c n) -> p c n", c=4),
                              wf_d[l].rearrange("c p n -> p c n"))
            if not fast:
                p12_t = sb.tile([R, 6 * D], f32, tag="p12", bufs=1,
                                name=f"p12_{l}")
                nc.sync.dma_start(p12_t.rearrange("p (g n) -> p g n", g=6),
                                  p12_d[l].rearrange("g p n -> p g n"))
                ln1w = p12_t[:, 0 * D: 1 * D]
                ln1b = p12_t[:, 1 * D: 2 * D]
                ln2w = p12_t[:, 2 * D: 3 * D]
                ln2b = p12_t[:, 3 * D: 4 * D]
                bo12 = p12_t[:, 4 * D: 5 * D]
                bf12 = p12_t[:, 5 * D: 6 * D]
            else:
                ln1w = ln1b = ln2w = ln2b = bo12 = bf12 = None

            def layer_norm(xin, wln, bln):
                st6 = sb.tile([R, 6], f32, tag="lnst", bufs=2)
                nc.vector.bn_stats(st6[:], xin)
                mv = sb.tile([R, 2], f32, tag="lnmv", bufs=2)
                nc.vector.bn_aggr(mv[:], st6[:])
                # rs = 1/sqrt(var) via quake bit-trick + 2 Newton iters on
                # Pool (avoids Act Sqrt/Ln so the exp table never reloads;
                # eps=1e-5 is negligible vs var~1)
                yq = sb.tile([R, 1], f32, tag="lnyq", bufs=2)
                yi = yq.bitcast(i32)
                nc.vector.tensor_scalar(yi, mv[:, 1:2].bitcast(i32), 1, 0,
                                        OP.arith_shift_right, OP.bitwise_xor)
                nc.vector.tensor_tensor(out=yi, in0=qmag[:], in1=yi,
                                        op=OP.subtract)
                sq = sb.tile([R, 1], f32, tag="lnsq", bufs=2)
                for _ in range(2):
                    nc.vector.tensor_tensor(out=sq[:], in0=yq[:], in1=yq[:],
                                            op=OP.mult)
                    nc.vector.tensor_tensor(out=sq[:], in0=sq[:],
                                            in1=mv[:, 1:2], op=OP.mult)
                    nc.vector.tensor_scalar(sq[:], sq[:], -0.5, 1.5,
                                            OP.mult, OP.add)
                    nc.vector.tensor_tensor(out=yq[:], in0=yq[:], in1=sq[:],
                                            op=OP.mult)
                hb = sb.tile([R, D], bf16, tag="lnhb", bufs=2)
                if fast:
                    nc.vector.tensor_scalar(hb[:], xin, mv[:, 0:1], yq[:],
                                            OP.subtract, OP.mult)
                    return hb
                hh = sb.tile([R, D], f32, tag="lnh", bufs=2)
                nc.vector.tensor_scalar(hh[:], xin, mv[:, 0:1], yq[:],
                                        OP.subtract, OP.mult)
                nc.vector.tensor_tensor(out=hh[:], in0=hh[:], in1=wln, op=OP.mult)
                nc.vector.tensor_tensor(out=hb[:], in0=hh[:], in1=bln, op=OP.add)
                return hb

            h1 = layer_norm(x[:], ln1w, ln1b)

            # ---- hT2all [128, 48] bf16: h^T, col = 12c + 3e + tq ----
            hT2all = sb.tile([128, 4 * R], bf16, tag="hT2", bufs=2,
                             name=f"hT2_{l}")
            for c in range(NPAIR):
                tp = ps.tile([128, R], bf16, tag="sm", bufs=3)
                nc.tensor.transpose(tp[:], h1[:, 128 * c: 128 * (c + 1)],
                                    i128[0:R, 0:R])
                nc.vector.tensor_copy(hT2all[:, 12 * c: 12 * (c + 1)], tp[:])

            # ---- QKV projections (one matmul each for Q^T, K^T) ----
            q2d = sb.tile([128, 512], f8, tag="q2d", bufs=2, name=f"q2d_{l}")
            nc.gpsimd.memset(q2d[:], 0.0)
            q2dv = q2d.rearrange("p (g e i m) -> p g e i m", g=2, e=BB, i=2)
            knT = sb.tile([128, 256], f8, tag="knT", bufs=2, name=f"knT_{l}")
            nc.gpsimd.memset(knT[:], 0.0)
            knv = knT.rearrange("p (e g i m) -> p e g i m", e=BB, g=2, i=2)

            qT2all = ps.tile([128, 4 * R], f32, tag="sm", bufs=3)
            nc.tensor.matmul(qT2all[:], wq2all[:, 128 * l: 128 * (l + 1)],
                             hT2all[:])
            qv = qT2all.rearrange("p (c e t) -> p c e t", c=4, e=BB)
            for c in range(NPAIR):
                g, i = c % 2, c // 2
                for hf in range(2):
                    nc.vector.tensor_copy(
                        q2dv[64 * hf: 64 * hf + 64, g, :, i,
                             6 * i + 3 * hf: 6 * i + 3 * hf + 3],
                        qv[64 * hf: 64 * hf + 64, c, :, :])
            kT2all = ps.tile([128, 4 * R], f32, tag="sm", bufs=3)
            nc.tensor.matmul(kT2all[:], wk2all[:, 128 * l: 128 * (l + 1)],
                             hT2all[:])
            kv = kT2all.rearrange("p (i g e t) -> p e g i t", i=2, g=2, e=BB)
            for i in range(2):
                nc.vector.tensor_copy(knv[:, :, :, i, 0:TN], kv[:, :, :, i, :])
            vn_ps = ps.tile([R, D], f32, tag="vn", bufs=1)
            for c in range(NPAIR):
                nc.tensor.matmul(vn_ps[:, 128 * c: 128 * (c + 1)],
                                 hT2all[:, 12 * c: 12 * (c + 1)],
                                 wv2all[:, 128 * l: 128 * (l + 1)])
            vnsb = sb.tile([R, D], f8, tag="vnsb", bufs=2)
            nc.scalar.copy(vnsb[:], vn_ps[:])
            Vn = [sb.tile([TN, D], f8, tag="Vn", bufs=8, name=f"Vn_{l}_{e}")
                  for e in range(BB)]
            for e in range(BB):
                nc.sync.dma_start(Vn[e][:], vnsb[3 * e: 3 * e + TN, :])

            # ---- new-token E, transposed: EnT[tn, 64e + 32g + m] ----
            EnT = ps.tile([16, 256], f32, tag="sm", bufs=3)
            nc.vector.memset(EnT[:], 0.0)
            for e in range(BB):
                for g in range(2):
                    nc.tensor.matmul(
                        EnT[0:16, 64 * e + 32 * g: 64 * e + 32 * g + 32],
                        knv[:, e, g, :, :], q2dv[:, g, e, :, :],
                        perf_mode=DR)
            nc.vector.tensor_tensor(out=EnT[:], in0=EnT[:], in1=negnT[:],
                                    op=OP.add)
            AnT = sb.tile([16, 256], f8, tag="AnT", bufs=2, name=f"AnT_{l}")
            nc.scalar.activation(AnT[:], EnT[:], AF.Exp, scale=ESC)

            # ---- attention per env, software-pipelined ----
            CTall = sb.tile([128, 64], f8, tag="CT", bufs=2, name=f"CT_{l}")
            nc.gpsimd.memset(CTall[:], 0.0)
            ctv = CTall.rearrange("p (g b m) -> p g b m", g=2, b=2)
            vFv = vFA.rearrange("p (e j g i b dd) -> p e j g i b dd",
                                e=BB, j=4, g=2, i=2, b=2)

            def emit_E(e):
                E = ps.tile([128, 512], f32, tag="eb", bufs=2,
                            name=f"E_{l}_{e}")
                for g in range(2):
                    for h2 in range(2):
                        rb = 32 * (2 * g + h2)
                        for i in range(2):
                            nc.tensor.matmul(
                                E[rb: rb + 32, 0:512],
                                q2dv[:, g, e, i, :],
                                ktA[:, 4096 * e + 1024 * (g + 2 * i) + 512 * h2:
                                    4096 * e + 1024 * (g + 2 * i) + 512 * (h2 + 1)],
                                start=(i == 0), stop=(i == 1),
                                tile_position=(0, rb))
                A4 = sb.tile([128, 512], bf16, tag="A4", bufs=2,
                             name=f"A4_{l}_{e}")
                acc = sb.tile([128, 1], f32, tag="acc", bufs=2)
                nc.scalar.activation(A4[:], E[:], AF.Exp, scale=ESC,
                                     accum_out=acc[:])
                # A^T: col = 128jj + 64g + 32i + m  (i = h2 = t-half)
                ATp = ps.tile([128, 512], bf16, tag="sm", bufs=3)
                for jj in range(4):
                    nc.tensor.transpose(ATp[:, 128 * jj: 128 * (jj + 1)],
                                        A4[:, 128 * jj: 128 * (jj + 1)],
                                        i128[:, :])
                AT = sb.tile([128, 512], f8, tag="AT", bufs=2,
                             name=f"AT_{l}_{e}")
                nc.scalar.copy(AT[:], ATp[:])
                ATv = AT.rearrange("p (j g i m) -> p j g i m", j=4, g=2, i=2)
                # denominators: fold h2 halves of acc (Act bias allows the
                # cross-partition-base read), add new-token sums, rcp on DVE
                dnew = ps.tile([32, 4], f32, tag="sm", bufs=3)
                for g in range(2):
                    nc.tensor.matmul(
                        dnew[0:32, 2 * g: 2 * g + 2],
                        AnT[0:TN, 64 * e + 32 * g: 64 * e + 32 * g + 32],
                        o2f8[0:TN, 0:2])
                ds = sb.tile([32, 2], f32, tag="ds", bufs=2)
                for g in range(2):
                    nc.scalar.activation(ds[:, g: g + 1],
                                         acc[64 * g: 64 * g + 32, :],
                                         AF.Identity,
                                         bias=acc[64 * g + 32: 64 * g + 64, :])
                rcp = sb.tile([32, 2], f32, tag="rcp", bufs=2)
                nc.vector.tensor_tensor(
                    out=rcp[:], in0=ds[:],
                    in1=dnew.rearrange("p (g o) -> p g o", g=2)[:, :, 0],
                    op=OP.add)
                nc.vector.tensor_tensor(out=rcp[:], in0=rcp[:],
                                        in1=npad4[:, 2 * e: 2 * e + 2],
                                        op=OP.subtract)
                nc.vector.reciprocal(rcp[:], rcp[:])
                return ATv, rcp

            def emit_attn(e, ATv, rcp):
                otp = ps.tile([128, 48], f32, tag="sm", bufs=3)
                for g in range(2):
                    og = ps.tile([32, 256], f32, tag="og", bufs=2,
                                 name=f"og_{l}_{e}_{g}")
                    for jp in range(4):
                        nc.tensor.matmul(
                            og[0:32, 0:256],
                            ATv[:, jp, g, :, :],
                            vFv[:, e, jp, g, :, :, :],
                            perf_mode=DR, start=(jp == 0), stop=False)
                    nc.tensor.matmul(
                        og[0:32, 0:256],
                        AnT[0:TN, 64 * e + 32 * g: 64 * e + 32 * g + 32],
                        Vn[e].rearrange("p (b d2) -> p b d2", b=2)[
                            :, :, 128 * g: 128 * (g + 1)],
                        start=False, stop=True)
                    onrm = sb.tile([32, 256], bf16, tag="onrm", bufs=4)
                    nc.vector.tensor_scalar_mul(onrm[:], og[:],
                                                rcp[:, g: g + 1])
                    # O^T gather: identity-slice matmuls select the 6 valid
                    # rows per (g, b); otp col = 12c + 3hf + tq
                    for b in range(2):
                        c = g + 2 * b
                        nc.tensor.matmul(
                            otp[:, 12 * c: 12 * c + 6],
                            onrm[:, 128 * b: 128 * (b + 1)],
                            i128[0:32, 6 * b: 6 * b + 6])
                src = otp.rearrange("p (b g u) -> p g b u", b=2, g=2)
                for hf in range(2):
                    nc.vector.tensor_copy(
                        ctv[64 * hf: 64 * hf + 64, :, :, 3 * e: 3 * e + 3],
                        src[64 * hf: 64 * hf + 64, :, :,
                            3 * hf: 3 * hf + 3])

            pend = emit_E(0)
            for e in range(BB):
                nxt = emit_E(e + 1) if e + 1 < BB else None
                emit_attn(e, *pend)
                pend = nxt

            # ---- output projection (fp8 DR over b-pairs) + residual ----
            xo = ps.tile([16, D], f32, tag="sm", bufs=3)
            wov = wo_t.rearrange("p (g b n) -> p g b n", g=2, b=2)
            for nh in range(2):
                for g in range(2):
                    nc.tensor.matmul(
                        xo[0:16, 256 * nh: 256 * (nh + 1)],
                        ctv[:, g, :, :],
                        wov[:, g, :, 256 * nh: 256 * (nh + 1)],
                        perf_mode=DR, start=(g == 0), stop=(g == 1))
            if fast:
                nc.vector.tensor_tensor(out=x[:], in0=x[:], in1=xo[0:R, :],
                                        op=OP.add)
            else:
                xt = sb.tile([R, D], f32, tag="scr", bufs=2)
                nc.vector.tensor_tensor(out=xt[:], in0=xo[0:R, :], in1=bo12,
                                        op=OP.add)
                nc.vector.tensor_tensor(out=x[:], in0=x[:], in1=xt[:], op=OP.add)

            # ---- FFN ----
            h2t = layer_norm(x[:], ln2w, ln2b)
            HTall = sb.tile([128, 4 * R], bf16, tag="HT", bufs=2,
                            name=f"HT_{l}")
            for c in range(NPAIR):
                tp = ps.tile([128, R], bf16, tag="sm", bufs=3)
                nc.tensor.transpose(tp[:], h2t[:, 128 * c: 128 * (c + 1)],
                                    i128[0:R, 0:R])
                nc.vector.tensor_copy(HTall[:, 12 * c: 12 * (c + 1)], tp[:])
            ff = ps.tile([R, D], f32, tag="sm", bufs=3)
            for c in range(NPAIR):
                nc.tensor.matmul(ff[:], HTall[:, 12 * c: 12 * (c + 1)],
                                 wf_t[:, c * D: (c + 1) * D],
                                 start=(c == 0), stop=(c == NPAIR - 1))
            ft = sb.tile([R, D], f32, tag="scr", bufs=2)
            if fast:
                nc.scalar.activation(ft[:], ff[:], AF.Relu)
            else:
                nc.vector.tensor_tensor(out=ft[:], in0=ff[:], in1=bf12, op=OP.add)
                nc.scalar.activation(ft[:], ft[:], AF.Relu)
            nc.vector.tensor_tensor(out=x[:], in0=x[:], in1=ft[:], op=OP.add)

        nc.sync.dma_start(out_d[:], x[:])

    nc.compile()
    return nc


def _prep_inputs(x, past_k, past_v, pad_mask, ln1_w, ln1_b, ln2_w, ln2_b,
                 Wq, Wk, Wv, Wo, bo, Wf, bf):
    import ml_dtypes
    f = np.float32
    b16 = ml_dtypes.bfloat16
    fp8 = ml_dtypes.float8_e4m3
    x = np.ascontiguousarray(x, f)
    past_k = np.asarray(past_k, f)
    past_v = np.asarray(past_v, f)
    pad_mask = np.asarray(pad_mask)

    def blk2(wT):
        out = np.zeros((L, 128, 128), f)
        out[:, 0:64, 0:64] = wT
        out[:, 64:128, 64:128] = wT
        return out.astype(b16)
    wq2 = blk2(np.transpose(np.asarray(Wq, f), (0, 2, 1)))
    wk2 = blk2(np.transpose(np.asarray(Wk, f), (0, 2, 1)))
    wv2 = blk2(np.transpose(np.asarray(Wv, f), (0, 2, 1)))
    woT = np.transpose(np.asarray(Wo, f), (0, 2, 1)).reshape(L, 2, 2, 128, D)
    # wo2[l, p, 1024g + 512b + n] = Wo^T[l, 128(g+2b) + p, n]; c = 2b + g in
    # woT's reshape is (b, g) so transpose to (l, p, g, b, n)
    wo2 = np.transpose(woT, (0, 3, 2, 1, 4)).reshape(L, 128, 2048)
    wo2 = np.clip(wo2, -240.0, 240.0).astype(fp8)
    wfT = np.transpose(np.asarray(Wf, f), (0, 2, 1)).reshape(L, 4, 128, D).astype(b16)
    p12 = np.stack(
        [np.broadcast_to(np.asarray(a, f)[:, None, :], (L, R, D))
         for a in (ln1_w, ln1_b, ln2_w, ln2_b, bo, bf)], axis=1)
    p12 = np.ascontiguousarray(p12)
    i128 = np.eye(128, dtype=b16)
    ones2 = np.ones((128, 32), f).astype(fp8)

    def to8(a):
        return np.clip(a, -240.0, 240.0).astype(fp8)

    in_maps = []
    for cc in range(NC):
        bs = slice(cc * BB, (cc + 1) * BB)
        pk = past_k[:, bs]                      # (L, BB, H, TP, Dh)
        pv = past_v[:, bs]
        # ktT[l, e, 64hf+d, 1024pr+t] = pk[l, e, 2pr+hf, t, d]
        kt = pk.reshape(L, BB, NPAIR, 2, TP, Dh)
        kt = np.transpose(kt, (0, 1, 3, 5, 2, 4))    # l, e, hf, d, pr, t
        kt = to8(np.ascontiguousarray(kt.reshape(L, BB, 128, NPAIR * TP)))
        # vF[l, e, p, 1024jp+512g+256i+128b+64hf+d] =
        #    pv[l, e, 4b+2g+hf, 512i+128jp+p, d]
        vf = pv.reshape(L, BB, 2, 2, 2, 2, NJ // 2, 128, Dh)
        #                      b  g  hf i  jp      p    d
        vf = np.transpose(vf, (0, 1, 7, 6, 3, 5, 2, 4, 8))
        #   -> l, e, p, jp, g, i, b, hf, d
        vf = to8(np.ascontiguousarray(vf.reshape(L, BB, 128, NPAIR * TP)))

        pm = np.asarray(pad_mask[bs])           # (BB, Tt) bool
        npad_e = (TP - pm[:, :TP].sum(axis=1)).astype(f)   # (BB,)

        npad4 = np.zeros((32, 2 * BB), f)
        for e in range(BB):
            npad4[0:12, 2 * e: 2 * e + 2] = npad_e[e]
        negnT = np.full((16, 256), NEG, f)
        for e in range(BB):
            for tn in range(TN):
                for g in range(2):
                    for m in range(12):
                        tq = m % 3
                        if tn <= tq and bool(pm[e, TP + tn]):
                            negnT[tn, 64 * e + 32 * g + m] = 0.0

        in_maps.append({
            "x0": np.ascontiguousarray(x[bs].reshape(R, D)),
            "ktT": kt, "vF": vf,
            "wq2": wq2, "wk2": wk2, "wv2": wv2,
            "wo2": wo2, "wfT": wfT, "p12": p12,
            "i128b": i128, "ones2f8": ones2,
            "negnT": negnT, "npad4": npad4,
        })
    return in_maps


_CACHE = {}


def kernel(**inputs):
    import os
    import sys
    for p in ("/opt/trn_rl_repo", "/opt/pypackages"):
        if p not in sys.path:
            sys.path.insert(0, p)
    os.environ.setdefault("JAX_PLATFORMS", "")
    from concourse.bass_utils import run_bass_kernel_spmd

    in_maps = _prep_inputs(**inputs)
    fast = all(np.allclose(np.asarray(inputs[k]), 1.0) for k in ("ln1_w", "ln2_w")) \
        and all(np.allclose(np.asarray(inputs[k]), 0.0)
                for k in ("ln1_b", "ln2_b", "bo", "bf"))
    key = f"nc_{fast}"
    if key not in _CACHE:
        _CACHE[key] = _build_bass(fast)
    nc = _CACHE[key]
    res = run_bass_kernel_spmd(nc, in_maps, core_ids=list(range(NC)))
    out = np.concatenate([r["xout"].reshape(BB, TN, D) for r in res.results], axis=0)
    return out.astype(np.float32)


# revision 26
# speedup vs baseline: 1.4115x; 1.0132x over previous
"""Trainium2 Bass kernel: 4-layer decode-attention transformer block (fp8 KV).

Shapes (hardcoded): L=4, B=32, H=8, Dh=64, D=512, TP=1024, TN=3, Tt=1027.
Sharding: data-parallel over B across 8 cores (4 envs each); params replicated.

v5 design notes (evolved from the 234.6us bf16 v2 baseline; v4 fp8 measured
210.9us, PE-bound on instruction count):
 - K/V/Wo streamed from HBM in fp8 e4m3 (~20.5MB/core vs 38.8MB).
 - E packed [128, 512] per env: row = 32*(2g + h2) + m, col = key t within
   the h2 half (m = 6i + 3hf + tq labels the two pair-blocks g+2i). This
   packing lets ONE Act exp call (512 cols) cover the whole env, with
   accum_out producing the softmax denominators for free.
 - QK^T is 8 plain fp8 matmuls [32,512] per env (DoubleRow outputs must
   start at partition 0 - walrus codegen constraint - so DR can't write the
   row-packed E); AV, denominator-fold tails, new-token E, and the Wo
   projection DO use fp8 DoubleRow with base-0 outputs.
 - A^T via 4 PE transposes [128,128] -> one scalar.copy; the AT column
   layout 128*jj + 64g + 32i + m falls out with i = h2 as the DoubleRow
   k-tile pair (t, t+512), matching the vF host layout.
 - Denominator: exp accum_out [128,1] folded pairwise by 2 Act Identity ops
   with cross-partition-base bias (engines allow that only via Act bias);
   plus tiny new-token ones-matmuls; rcp on DVE.
 - O gather: plain matmuls with identity slices select the 6 valid rows per
   (g,b) out of onrm^T, so the CT gather is 2 strided copies (on Pool).
 - Pool engine (nc.gpsimd) offloads residual adds, copies, quake rsqrt.
 - LN: DVE bn_stats/bn_aggr; 1/sqrt(var) via quake bit-trick + 2 Newton
   iterations on Pool (keeps Act tables pinned to the exp set).
"""

import numpy as np

L, B, H, Dh, D, TP, TN = 4, 32, 8, 64, 512, 1024, 3
Tt = TP + TN
NC = 8
BB = B // NC          # envs per core = 4
R = BB * TN           # x rows per core = 12
NJ = TP // 128        # t-chunks of 128 = 8
NPAIR = H // 2        # head pairs = 4
EPS = 1e-5
NEG = -1e9
ESC = 0.125           # 1/sqrt(Dh), applied as Act scale at exp time
QMAGIC = 1597463007.0  # 0x5f3759df


def _build_bass(fast=True):
    import concourse.bass as bass
    import concourse.mybir as mybir
    import concourse.tile as tile
    from concourse import bacc

    f32 = mybir.dt.float32
    i32 = mybir.dt.int32
    bf16 = mybir.dt.bfloat16
    f8 = mybir.dt.float8e4
    AF = mybir.ActivationFunctionType
    OP = mybir.AluOpType
    DR = mybir.MatmulPerfMode.DoubleRow

    nc = bacc.Bacc("TRN2", target_bir_lowering=False, debug=False, num_devices=NC)

    x_d = nc.dram_tensor("x0", [R, D], f32, kind="ExternalInput")
    # K^T per (l, env): rows 64*hf+d, cols 1024*pr + t
    kt_d = nc.dram_tensor("ktT", [L, BB, 128, NPAIR * TP], f8, kind="ExternalInput")
    # V per (l, env): rows p, cols 1024*jp + 512*g + 256*i + 128*b + 64*hf + d
    #  = V[head 4b+2g+hf, t = 512*i + 128*jp + p, d]
    vf_d = nc.dram_tensor("vF", [L, BB, 128, NPAIR * TP], f8, kind="ExternalInput")
    wq_d = nc.dram_tensor("wq2", [L, 128, 128], bf16, kind="ExternalInput")
    wk_d = nc.dram_tensor("wk2", [L, 128, 128], bf16, kind="ExternalInput")
    wv_d = nc.dram_tensor("wv2", [L, 128, 128], bf16, kind="ExternalInput")
    # Wo as fp8 DR pairs: col = 1024g + 512b + n  (c = g + 2b)
    wo_d = nc.dram_tensor("wo2", [L, 128, 2 * D * 2], f8, kind="ExternalInput")
    wf_d = nc.dram_tensor("wfT", [L, 4, 128, D], bf16, kind="ExternalInput")
    p12_d = nc.dram_tensor("p12", [L, 6, R, D], f32, kind="ExternalInput")
    i128_d = nc.dram_tensor("i128b", [128, 128], bf16, kind="ExternalInput")
    ones_d = nc.dram_tensor("ones2f8", [128, 32], f8, kind="ExternalInput")
    # new-token causal/pad bias, rows tn, cols 64e + 32g + m
    negnT_d = nc.dram_tensor("negnT", [16, 256], f32, kind="ExternalInput")
    # padded-slot count, rows m, col 2e + g
    npad_d = nc.dram_tensor("npad4", [32, 2 * BB], f32, kind="ExternalInput")
    out_d = nc.dram_tensor("xout", [R, D], f32, kind="ExternalOutput")

    from contextlib import ExitStack

    with tile.TileContext(nc) as tc, ExitStack() as st:
        consts = st.enter_context(tc.tile_pool(name="consts", bufs=1))
        sb = st.enter_context(tc.tile_pool(name="sb", bufs=1))
        ps = st.enter_context(tc.tile_pool(name="ps", bufs=1, space="PSUM"))

        x = consts.tile([R, D], f32)
        nc.sync.dma_start(x[:], x_d[:])
        i128 = consts.tile([128, 128], bf16)
        nc.sync.dma_start(i128[:], i128_d[:])
        o2f8 = consts.tile([128, 32], f8)
        nc.sync.dma_start(o2f8[:], ones_d[:])
        negnT = consts.tile([16, 256], f32)
        nc.sync.dma_start(negnT[:], negnT_d[:])
        npad4 = consts.tile([32, 2 * BB], f32)
        nc.sync.dma_start(npad4[:], npad_d[:])
        qmag = consts.tile([R, 1], i32)
        nc.vector.memset(qmag[:], QMAGIC)

        wq2all = consts.tile([128, L * 128], bf16)
        nc.sync.dma_start(wq2all.rearrange("p (l n) -> p l n", l=L),
                          wq_d.rearrange("l p n -> p l n"))
        wk2all = consts.tile([128, L * 128], bf16)
        nc.sync.dma_start(wk2all.rearrange("p (l n) -> p l n", l=L),
                          wk_d.rearrange("l p n -> p l n"))
        wv2all = consts.tile([128, L * 128], bf16)
        nc.sync.dma_start(wv2all.rearrange("p (l n) -> p l n", l=L),
                          wv_d.rearrange("l p n -> p l n"))

        for l in range(L):
            # ---- per-layer loads ----
            ktA = sb.tile([128, BB * NPAIR * TP], f8, tag="ktA", bufs=2,
                          name=f"ktA_{l}")
            nc.sync.dma_start(ktA.rearrange("p (e n) -> p e n", e=BB),
                              kt_d[l].rearrange("e p n -> p e n"))
            vFA = sb.tile([128, BB * NPAIR * TP], f8, tag="vFA", bufs=2,
                          name=f"vFA_{l}")
            nc.sync.dma_start(vFA.rearrange("p (e n) -> p e n", e=BB),
                              vf_d[l].rearrange("e p n -> p e n"))
            wo_t = sb.tile([128, 4 * D], f8, tag="wo", bufs=2, name=f"wo_{l}")
            nc.sync.dma_start(wo_t[:], wo_d[l])
            wf_t = sb.tile([128, 4 * D], bf16, tag="wf", bufs=2, name=f"wf_{l}")
            nc.sync.dma_start(wf_t.rearrange("p (c n) -> p c n", c=4),
                              wf_d[l].rearrange("c p n -> p c n"))
            if not fast:
                p12_t = sb.tile([R, 6 * D], f32, tag="p12", bufs=1,
                                name=f"p12_{l}")
                nc.sync.dma_start(p12_t.rearrange("p (g n) -> p g n", g=6),
                                  p12_d[l].rearrange("g p n -> p g n"))
                ln1w = p12_t[:, 0 * D: 1 * D]
                ln1b = p12_t[:, 1 * D: 2 * D]
                ln2w = p12_t[:, 2 * D: 3 * D]
                ln2b = p12_t[:, 3 * D: 4 * D]
                bo12 = p12_t[:, 4 * D: 5 * D]
                bf12 = p12_t[:, 5 * D: 6 * D]
            else:
                ln1w = ln1b = ln2w = ln2b = bo12 = bf12 = None

            def layer_norm(xin, wln, bln):
                st6 = sb.tile([R, 6], f32, tag="lnst", bufs=2)
                nc.vector.bn_stats(st6[:], xin)
                mv = sb.tile([R, 2], f32, tag="lnmv", bufs=2)
                nc.vector.bn_aggr(mv[:], st6[:])
                # rs = 1/sqrt(var) via quake bit-trick + 2 Newton iters on
                # Pool (avoids Act Sqrt/Ln so the exp table never reloads;
                # eps=1e-5 is negligible vs var~1)
                yq = sb.tile([R, 1], f32, tag="lnyq", bufs=2)
                yi = yq.bitcast(i32)
                nc.vector.tensor_scalar(yi, mv[:, 1:2].bitcast(i32), 1, 0,
                                        OP.arith_shift_right, OP.bitwise_xor)
                nc.vector.tensor_tensor(out=yi, in0=qmag[:], in1=yi,
                                        op=OP.subtract)
                sq = sb.tile([R, 1], f32, tag="lnsq", bufs=2)
                for _ in range(1):
                    nc.vector.tensor_tensor(out=sq[:], in0=yq[:], in1=yq[:],
                                            op=OP.mult)
                    nc.vector.tensor_tensor(out=sq[:], in0=sq[:],
                                            in1=mv[:, 1:2], op=OP.mult)
                    nc.vector.tensor_scalar(sq[:], sq[:], -0.5, 1.5,
                                            OP.mult, OP.add)
                    nc.vector.tensor_tensor(out=yq[:], in0=yq[:], in1=sq[:],
                                            op=OP.mult)
                hb = sb.tile([R, D], bf16, tag="lnhb", bufs=2)
                if fast:
                    nc.vector.tensor_scalar(hb[:], xin, mv[:, 0:1], yq[:],
                                            OP.subtract, OP.mult)
                    return hb
                hh = sb.tile([R, D], f32, tag="lnh", bufs=2)
                nc.vector.tensor_scalar(hh[:], xin, mv[:, 0:1], yq[:],
                                        OP.subtract, OP.mult)
                nc.vector.tensor_tensor(out=hh[:], in0=hh[:], in1=wln, op=OP.mult)
                nc.vector.tensor_tensor(out=hb[:], in0=hh[:], in1=bln, op=OP.add)
                return hb

            h1 = layer_norm(x[:], ln1w, ln1b)

            # ---- hT2all [128, 48] bf16: h^T, col = 12c + 3e + tq ----
            hT2all = sb.tile([128, 4 * R], bf16, tag="hT2", bufs=2,
                             name=f"hT2_{l}")
            tp1 = ps.tile([128, 4 * R], bf16, tag="sm", bufs=3)
            for c in range(NPAIR):
                nc.tensor.transpose(tp1[:, 12 * c: 12 * (c + 1)],
                                    h1[:, 128 * c: 128 * (c + 1)],
                                    i128[0:R, 0:R])
            nc.vector.tensor_copy(hT2all[:], tp1[:])

            # ---- QKV projections (one matmul each for Q^T, K^T) ----
            q2d = sb.tile([128, 512], f8, tag="q2d", bufs=2, name=f"q2d_{l}")
            nc.gpsimd.memset(q2d[:], 0.0)
            q2dv = q2d.rearrange("p (g e i m) -> p g e i m", g=2, e=BB, i=2)
            knT = sb.tile([128, 256], f8, tag="knT", bufs=2, name=f"knT_{l}")
            nc.gpsimd.memset(knT[:], 0.0)
            knv = knT.rearrange("p (e g i m) -> p e g i m", e=BB, g=2, i=2)

            qT2all = ps.tile([128, 4 * R], f32, tag="sm", bufs=3)
            nc.tensor.matmul(qT2all[:], wq2all[:, 128 * l: 128 * (l + 1)],
                             hT2all[:])
            qv = qT2all.rearrange("p (c e t) -> p c e t", c=4, e=BB)
            for c in range(NPAIR):
                g, i = c % 2, c // 2
                for hf in range(2):
                    nc.vector.tensor_copy(
                        q2dv[64 * hf: 64 * hf + 64, g, :, i,
                             6 * i + 3 * hf: 6 * i + 3 * hf + 3],
                        qv[64 * hf: 64 * hf + 64, c, :, :])
            kT2all = ps.tile([128, 4 * R], f32, tag="sm", bufs=3)
            nc.tensor.matmul(kT2all[:], wk2all[:, 128 * l: 128 * (l + 1)],
                             hT2all[:])
            kv = kT2all.rearrange("p (i g e t) -> p e g i t", i=2, g=2, e=BB)
            for i in range(2):
                nc.vector.tensor_copy(knv[:, :, :, i, 0:TN], kv[:, :, :, i, :])
            vn_ps = ps.tile([R, D], f32, tag="vn", bufs=1)
            for c in range(NPAIR):
                nc.tensor.matmul(vn_ps[:, 128 * c: 128 * (c + 1)],
                                 hT2all[:, 12 * c: 12 * (c + 1)],
                                 wv2all[:, 128 * l: 128 * (l + 1)])
            vnsb = sb.tile([R, D], f8, tag="vnsb", bufs=2)
            nc.scalar.copy(vnsb[:], vn_ps[:])
            Vn = [sb.tile([TN, D], f8, tag="Vn", bufs=8, name=f"Vn_{l}_{e}")
                  for e in range(BB)]
            for e in range(BB):
                nc.sync.dma_start(Vn[e][:], vnsb[3 * e: 3 * e + TN, :])

            # ---- new-token E, transposed: EnT[tn, 64e + 32g + m] ----
            EnT = ps.tile([16, 256], f32, tag="sm", bufs=3)
            nc.vector.memset(EnT[:], 0.0)
            for e in range(BB):
                for g in range(2):
                    nc.tensor.matmul(
                        EnT[0:16, 64 * e + 32 * g: 64 * e + 32 * g + 32],
                        knv[:, e, g, :, :], q2dv[:, g, e, :, :],
                        perf_mode=DR)
            nc.vector.tensor_tensor(out=EnT[:], in0=EnT[:], in1=negnT[:],
                                    op=OP.add)
            AnT = sb.tile([16, 256], f8, tag="AnT", bufs=2, name=f"AnT_{l}")
            nc.scalar.activation(AnT[:], EnT[:], AF.Exp, scale=ESC)

            # ---- attention per env, software-pipelined ----
            CTall = sb.tile([128, 64], f8, tag="CT", bufs=2, name=f"CT_{l}")
            nc.gpsimd.memset(CTall[:], 0.0)
            ctv = CTall.rearrange("p (g b m) -> p g b m", g=2, b=2)
            vFv = vFA.rearrange("p (e j g i b dd) -> p e j g i b dd",
                                e=BB, j=4, g=2, i=2, b=2)

            def emit_E(e):
                E = ps.tile([128, 512], f32, tag="eb", bufs=2,
                            name=f"E_{l}_{e}")
                for g in range(2):
                    for h2 in range(2):
                        rb = 32 * (2 * g + h2)
                        for i in range(2):
                            nc.tensor.matmul(
                                E[rb: rb + 32, 0:512],
                                q2dv[:, g, e, i, :],
                                ktA[:, 4096 * e + 1024 * (g + 2 * i) + 512 * h2:
                                    4096 * e + 1024 * (g + 2 * i) + 512 * (h2 + 1)],
                                start=(i == 0), stop=(i == 1),
                                tile_position=(0, rb))
                A4 = sb.tile([128, 512], bf16, tag="A4", bufs=2,
                             name=f"A4_{l}_{e}")
                acc = sb.tile([128, 1], f32, tag="acc", bufs=2)
                nc.scalar.activation(A4[:], E[:], AF.Exp, scale=ESC,
                                     accum_out=acc[:])
                # A^T: col = 128jj + 64g + 32i + m  (i = h2 = t-half)
                ATp = ps.tile([128, 512], bf16, tag="sm", bufs=3)
                for jj in range(4):
                    nc.tensor.transpose(ATp[:, 128 * jj: 128 * (jj + 1)],
                                        A4[:, 128 * jj: 128 * (jj + 1)],
                                        i128[:, :])
                AT = sb.tile([128, 512], f8, tag="AT", bufs=2,
                             name=f"AT_{l}_{e}")
                nc.scalar.copy(AT[:], ATp[:])
                ATv = AT.rearrange("p (j g i m) -> p j g i m", j=4, g=2, i=2)
                # denominators: fold h2 halves of acc (Act bias allows the
                # cross-partition-base read), add new-token sums, rcp on DVE
                dnew = ps.tile([32, 4], f32, tag="sm", bufs=3)
                for g in range(2):
                    nc.tensor.matmul(
                        dnew[0:32, 2 * g: 2 * g + 2],
                        AnT[0:TN, 64 * e + 32 * g: 64 * e + 32 * g + 32],
                        o2f8[0:TN, 0:2])
                ds = sb.tile([32, 2], f32, tag="ds", bufs=2)
                for g in range(2):
                    nc.scalar.activation(ds[:, g: g + 1],
                                         acc[64 * g: 64 * g + 32, :],
                                         AF.Identity,
                                         bias=acc[64 * g + 32: 64 * g + 64, :])
                rcp = sb.tile([32, 2], f32, tag="rcp", bufs=2)
                nc.vector.tensor_tensor(
                    out=rcp[:], in0=ds[:],
                    in1=dnew.rearrange("p (g o) -> p g o", g=2)[:, :, 0],
                    op=OP.add)
                nc.vector.tensor_tensor(out=rcp[:], in0=rcp[:],
                                        in1=npad4[:, 2 * e: 2 * e + 2],
                                        op=OP.subtract)
                nc.vector.reciprocal(rcp[:], rcp[:])
                return ATv, rcp

            def emit_attn(e, ATv, rcp):
                otp = ps.tile([128, 48], f32, tag="sm", bufs=3)
                for g in range(2):
                    og = ps.tile([32, 256], f32, tag="og", bufs=2,
                                 name=f"og_{l}_{e}_{g}")
                    for jp in range(4):
                        nc.tensor.matmul(
                            og[0:32, 0:256],
                            ATv[:, jp, g, :, :],
                            vFv[:, e, jp, g, :, :, :],
                            perf_mode=DR, start=(jp == 0), stop=False)
                    nc.tensor.matmul(
                        og[0:32, 0:256],
                        AnT[0:TN, 64 * e + 32 * g: 64 * e + 32 * g + 32],
                        Vn[e].rearrange("p (b d2) -> p b d2", b=2)[
                            :, :, 128 * g: 128 * (g + 1)],
                        start=False, stop=True)
                    onrm = sb.tile([32, 256], bf16, tag="onrm", bufs=4)
                    nc.vector.tensor_scalar_mul(onrm[:], og[:],
                                                rcp[:, g: g + 1])
                    # O^T gather: identity-slice matmuls select the 6 valid
                    # rows per (g, b); otp col = 12c + 3hf + tq
                    for b in range(2):
                        c = g + 2 * b
                        nc.tensor.matmul(
                            otp[:, 12 * c: 12 * c + 6],
                            onrm[:, 128 * b: 128 * (b + 1)],
                            i128[0:32, 6 * b: 6 * b + 6])
                src = otp.rearrange("p (b g u) -> p g b u", b=2, g=2)
                for hf in range(2):
                    nc.vector.tensor_copy(
                        ctv[64 * hf: 64 * hf + 64, :, :, 3 * e: 3 * e + 3],
                        src[64 * hf: 64 * hf + 64, :, :,
                            3 * hf: 3 * hf + 3])

            pend = emit_E(0)
            for e in range(BB):
                nxt = emit_E(e + 1) if e + 1 < BB else None
                emit_attn(e, *pend)
                pend = nxt

            # ---- output projection (fp8 DR over b-pairs) + residual ----
            xo = ps.tile([16, D], f32, tag="sm", bufs=3)
            wov = wo_t.rearrange("p (g b n) -> p g b n", g=2, b=2)
            for nh in range(2):
                for g in range(2):
                    nc.tensor.matmul(
                        xo[0:16, 256 * nh: 256 * (nh + 1)],
                        ctv[:, g, :, :],
                        wov[:, g, :, 256 * nh: 256 * (nh + 1)],
                        perf_mode=DR, start=(g == 0), stop=(g == 1))
            if fast:
                nc.vector.tensor_tensor(out=x[:], in0=x[:], in1=xo[0:R, :],
                                        op=OP.add)
            else:
                xt = sb.tile([R, D], f32, tag="scr", bufs=2)
                nc.vector.tensor_tensor(out=xt[:], in0=xo[0:R, :], in1=bo12,
                                        op=OP.add)
                nc.vector.tensor_tensor(out=x[:], in0=x[:], in1=xt[:], op=OP.add)

            # ---- FFN ----
            h2t = layer_norm(x[:], ln2w, ln2b)
            HTall = sb.tile([128, 4 * R], bf16, tag="HT", bufs=2,
                            name=f"HT_{l}")
            tp2 = ps.tile([128, 4 * R], bf16, tag="sm", bufs=3)
            for c in range(NPAIR):
                nc.tensor.transpose(tp2[:, 12 * c: 12 * (c + 1)],
                                    h2t[:, 128 * c: 128 * (c + 1)],
                                    i128[0:R, 0:R])
            nc.vector.tensor_copy(HTall[:], tp2[:])
            ff = ps.tile([R, D], f32, tag="sm", bufs=3)
            for c in range(NPAIR):
                nc.tensor.matmul(ff[:], HTall[:, 12 * c: 12 * (c + 1)],
                                 wf_t[:, c * D: (c + 1) * D],
                                 start=(c == 0), stop=(c == NPAIR - 1))
            ft = sb.tile([R, D], f32, tag="scr", bufs=2)
            if fast:
                nc.scalar.activation(ft[:], ff[:], AF.Relu)
            else:
                nc.vector.tensor_tensor(out=ft[:], in0=ff[:], in1=bf12, op=OP.add)
                nc.scalar.activation(ft[:], ft[:], AF.Relu)
            nc.vector.tensor_tensor(out=x[:], in0=x[:], in1=ft[:], op=OP.add)

        nc.sync.dma_start(out_d[:], x[:])

    nc.compile()
    return nc


def _prep_inputs(x, past_k, past_v, pad_mask, ln1_w, ln1_b, ln2_w, ln2_b,
                 Wq, Wk, Wv, Wo, bo, Wf, bf):
    import ml_dtypes
    f = np.float32
    b16 = ml_dtypes.bfloat16
    fp8 = ml_dtypes.float8_e4m3
    x = np.ascontiguousarray(x, f)
    past_k = np.asarray(past_k, f)
    past_v = np.asarray(past_v, f)
    pad_mask = np.asarray(pad_mask)

    def blk2(wT):
        out = np.zeros((L, 128, 128), f)
        out[:, 0:64, 0:64] = wT
        out[:, 64:128, 64:128] = wT
        return out.astype(b16)
    wq2 = blk2(np.transpose(np.asarray(Wq, f), (0, 2, 1)))
    wk2 = blk2(np.transpose(np.asarray(Wk, f), (0, 2, 1)))
    wv2 = blk2(np.transpose(np.asarray(Wv, f), (0, 2, 1)))
    woT = np.transpose(np.asarray(Wo, f), (0, 2, 1)).reshape(L, 2, 2, 128, D)
    # wo2[l, p, 1024g + 512b + n] = Wo^T[l, 128(g+2b) + p, n]; c = 2b + g in
    # woT's reshape is (b, g) so transpose to (l, p, g, b, n)
    wo2 = np.transpose(woT, (0, 3, 2, 1, 4)).reshape(L, 128, 2048)
    wo2 = np.clip(wo2, -240.0, 240.0).astype(fp8)
    wfT = np.transpose(np.asarray(Wf, f), (0, 2, 1)).reshape(L, 4, 128, D).astype(b16)
    p12 = np.stack(
        [np.broadcast_to(np.asarray(a, f)[:, None, :], (L, R, D))
         for a in (ln1_w, ln1_b, ln2_w, ln2_b, bo, bf)], axis=1)
    p12 = np.ascontiguousarray(p12)
    i128 = np.eye(128, dtype=b16)
    ones2 = np.ones((128, 32), f).astype(fp8)

    def to8(a):
        return np.clip(a, -240.0, 240.0).astype(fp8)

    in_maps = []
    for cc in range(NC):
        bs = slice(cc * BB, (cc + 1) * BB)
        pk = past_k[:, bs]                      # (L, BB, H, TP, Dh)
        pv = past_v[:, bs]
        # ktT[l, e, 64hf+d, 1024pr+t] = pk[l, e, 2pr+hf, t, d]
        kt = pk.reshape(L, BB, NPAIR, 2, TP, Dh)
        kt = np.transpose(kt, (0, 1, 3, 5, 2, 4))    # l, e, hf, d, pr, t
        kt = to8(np.ascontiguousarray(kt.reshape(L, BB, 128, NPAIR * TP)))
        # vF[l, e, p, 1024jp+512g+256i+128b+64hf+d] =
        #    pv[l, e, 4b+2g+hf, 512i+128jp+p, d]
        vf = pv.reshape(L, BB, 2, 2, 2, 2, NJ // 2, 128, Dh)
        #                      b  g  hf i  jp      p    d
        vf = np.transpose(vf, (0, 1, 7, 6, 3, 5, 2, 4, 8))
        #   -> l, e, p, jp, g, i, b, hf, d
        vf = to8(np.ascontiguousarray(vf.reshape(L, BB, 128, NPAIR * TP)))

        pm = np.asarray(pad_mask[bs])           # (BB, Tt) bool
        npad_e = (TP - pm[:, :TP].sum(axis=1)).astype(f)   # (BB,)

        npad4 = np.zeros((32, 2 * BB), f)
        for e in range(BB):
            npad4[0:12, 2 * e: 2 * e + 2] = npad_e[e]
        negnT = np.full((16, 256), NEG, f)
        for e in range(BB):
            for tn in range(TN):
                for g in range(2):
                    for m in range(12):
                        tq = m % 3
                        if tn <= tq and bool(pm[e, TP + tn]):
                            negnT[tn, 64 * e + 32 * g + m] = 0.0

        in_maps.append({
            "x0": np.ascontiguousarray(x[bs].reshape(R, D)),
            "ktT": kt, "vF": vf,
            "wq2": wq2, "wk2": wk2, "wv2": wv2,
            "wo2": wo2, "wfT": wfT, "p12": p12,
            "i128b": i128, "ones2f8": ones2,
            "negnT": negnT, "npad4": npad4,
        })
    return in_maps


_CACHE = {}


def kernel(**inputs):
    import os
    import sys
    for p in ("/opt/trn_rl_repo", "/opt/pypackages"):
        if p not in sys.path:
            sys.path.insert(0, p)
    os.environ.setdefault("JAX_PLATFORMS", "")
    from concourse.bass_utils import run_bass_kernel_spmd

    in_maps = _prep_inputs(**inputs)
    fast = all(np.allclose(np.asarray(inputs[k]), 1.0) for k in ("ln1_w", "ln2_w")) \
        and all(np.allclose(np.asarray(inputs[k]), 0.0)
                for k in ("ln1_b", "ln2_b", "bo", "bf"))
    key = f"nc_{fast}"
    if key not in _CACHE:
        _CACHE[key] = _build_bass(fast)
    nc = _CACHE[key]
    res = run_bass_kernel_spmd(nc, in_maps, core_ids=list(range(NC)))
    out = np.concatenate([r["xout"].reshape(BB, TN, D) for r in res.results], axis=0)
    return out.astype(np.float32)


# revision 27
# speedup vs baseline: 1.4225x; 1.0077x over previous
"""Trainium2 Bass kernel: 4-layer decode-attention transformer block (fp8 KV).

Shapes (hardcoded): L=4, B=32, H=8, Dh=64, D=512, TP=1024, TN=3, Tt=1027.
Sharding: data-parallel over B across 8 cores (4 envs each); params replicated.

v5 design notes (evolved from the 234.6us bf16 v2 baseline; v4 fp8 measured
210.9us, PE-bound on instruction count):
 - K/V/Wo streamed from HBM in fp8 e4m3 (~20.5MB/core vs 38.8MB).
 - E packed [128, 512] per env: row = 32*(2g + h2) + m, col = key t within
   the h2 half (m = 6i + 3hf + tq labels the two pair-blocks g+2i). This
   packing lets ONE Act exp call (512 cols) cover the whole env, with
   accum_out producing the softmax denominators for free.
 - QK^T is 8 plain fp8 matmuls [32,512] per env (DoubleRow outputs must
   start at partition 0 - walrus codegen constraint - so DR can't write the
   row-packed E); AV, denominator-fold tails, new-token E, and the Wo
   projection DO use fp8 DoubleRow with base-0 outputs.
 - A^T via 4 PE transposes [128,128] -> one scalar.copy; the AT column
   layout 128*jj + 64g + 32i + m falls out with i = h2 as the DoubleRow
   k-tile pair (t, t+512), matching the vF host layout.
 - Denominator: exp accum_out [128,1] folded pairwise by 2 Act Identity ops
   with cross-partition-base bias (engines allow that only via Act bias);
   plus tiny new-token ones-matmuls; rcp on DVE.
 - O gather: plain matmuls with identity slices select the 6 valid rows per
   (g,b) out of onrm^T, so the CT gather is 2 strided copies (on Pool).
 - Pool engine (nc.gpsimd) offloads residual adds, copies, quake rsqrt.
 - LN: DVE bn_stats/bn_aggr; 1/sqrt(var) via quake bit-trick + 2 Newton
   iterations on Pool (keeps Act tables pinned to the exp set).
"""

import numpy as np

L, B, H, Dh, D, TP, TN = 4, 32, 8, 64, 512, 1024, 3
Tt = TP + TN
NC = 8
BB = B // NC          # envs per core = 4
R = BB * TN           # x rows per core = 12
NJ = TP // 128        # t-chunks of 128 = 8
NPAIR = H // 2        # head pairs = 4
EPS = 1e-5
NEG = -1e9
ESC = 0.125           # 1/sqrt(Dh), applied as Act scale at exp time
QMAGIC = 1597463007.0  # 0x5f3759df


def _build_bass(fast=True):
    import concourse.bass as bass
    import concourse.mybir as mybir
    import concourse.tile as tile
    from concourse import bacc

    f32 = mybir.dt.float32
    i32 = mybir.dt.int32
    bf16 = mybir.dt.bfloat16
    f8 = mybir.dt.float8e4
    AF = mybir.ActivationFunctionType
    OP = mybir.AluOpType
    DR = mybir.MatmulPerfMode.DoubleRow
    AX = mybir.AxisListType

    nc = bacc.Bacc("TRN2", target_bir_lowering=False, debug=False, num_devices=NC)

    x_d = nc.dram_tensor("x0", [R, D], f32, kind="ExternalInput")
    # K^T per (l, env): rows 64*hf+d, cols 1024*pr + t
    kt_d = nc.dram_tensor("ktT", [L, BB, 128, NPAIR * TP], f8, kind="ExternalInput")
    # V per (l, env): rows p, cols 1024*jp + 512*g + 256*i + 128*b + 64*hf + d
    #  = V[head 4b+2g+hf, t = 512*i + 128*jp + p, d]
    vf_d = nc.dram_tensor("vF", [L, BB, 128, NPAIR * TP], f8, kind="ExternalInput")
    wq_d = nc.dram_tensor("wq2", [L, 128, 128], bf16, kind="ExternalInput")
    wk_d = nc.dram_tensor("wk2", [L, 128, 128], bf16, kind="ExternalInput")
    wv_d = nc.dram_tensor("wv2", [L, 128, 128], bf16, kind="ExternalInput")
    # Wo as fp8 DR pairs: col = 1024g + 512b + n  (c = g + 2b)
    wo_d = nc.dram_tensor("wo2", [L, 128, 2 * D * 2], f8, kind="ExternalInput")
    wf_d = nc.dram_tensor("wfT", [L, 4, 128, D], bf16, kind="ExternalInput")
    p12_d = nc.dram_tensor("p12", [L, 6, R, D], f32, kind="ExternalInput")
    i128_d = nc.dram_tensor("i128b", [128, 128], bf16, kind="ExternalInput")
    ones_d = nc.dram_tensor("ones2f8", [128, 32], f8, kind="ExternalInput")
    # new-token causal/pad bias, rows tn, cols 64e + 32g + m
    negnT_d = nc.dram_tensor("negnT", [16, 256], f32, kind="ExternalInput")
    # padded-slot count, rows m, col 2e + g
    npad_d = nc.dram_tensor("npad4", [32, 2 * BB], f32, kind="ExternalInput")
    out_d = nc.dram_tensor("xout", [R, D], f32, kind="ExternalOutput")

    from contextlib import ExitStack

    with tile.TileContext(nc) as tc, ExitStack() as st:
        consts = st.enter_context(tc.tile_pool(name="consts", bufs=1))
        sb = st.enter_context(tc.tile_pool(name="sb", bufs=1))
        ps = st.enter_context(tc.tile_pool(name="ps", bufs=1, space="PSUM"))

        x = consts.tile([R, D], f32)
        nc.sync.dma_start(x[:], x_d[:])
        i128 = consts.tile([128, 128], bf16)
        nc.sync.dma_start(i128[:], i128_d[:])
        o2f8 = consts.tile([128, 32], f8)
        nc.sync.dma_start(o2f8[:], ones_d[:])
        negnT = consts.tile([16, 256], f32)
        nc.sync.dma_start(negnT[:], negnT_d[:])
        npad4 = consts.tile([32, 2 * BB], f32)
        nc.sync.dma_start(npad4[:], npad_d[:])
        qmag = consts.tile([R, 1], i32)
        nc.vector.memset(qmag[:], QMAGIC)

        s1n = consts.tile([R, 1], f32)
        nc.vector.tensor_reduce(s1n[:], x[:], AX.X, OP.add)

        wq2all = consts.tile([128, L * 128], bf16)
        nc.sync.dma_start(wq2all.rearrange("p (l n) -> p l n", l=L),
                          wq_d.rearrange("l p n -> p l n"))
        wk2all = consts.tile([128, L * 128], bf16)
        nc.sync.dma_start(wk2all.rearrange("p (l n) -> p l n", l=L),
                          wk_d.rearrange("l p n -> p l n"))
        wv2all = consts.tile([128, L * 128], bf16)
        nc.sync.dma_start(wv2all.rearrange("p (l n) -> p l n", l=L),
                          wv_d.rearrange("l p n -> p l n"))

        for l in range(L):
            # ---- per-layer loads ----
            ktA = sb.tile([128, BB * NPAIR * TP], f8, tag="ktA", bufs=2,
                          name=f"ktA_{l}")
            for e in range(BB):
                nc.sync.dma_start(ktA[:, 4096 * e: 4096 * (e + 1)],
                                  kt_d[l, e])
            vFA = sb.tile([128, BB * NPAIR * TP], f8, tag="vFA", bufs=2,
                          name=f"vFA_{l}")
            nc.sync.dma_start(vFA.rearrange("p (e n) -> p e n", e=BB),
                              vf_d[l].rearrange("e p n -> p e n"))
            wo_t = sb.tile([128, 4 * D], f8, tag="wo", bufs=2, name=f"wo_{l}")
            nc.sync.dma_start(wo_t[:], wo_d[l])
            wf_t = sb.tile([128, 4 * D], bf16, tag="wf", bufs=2, name=f"wf_{l}")
            nc.sync.dma_start(wf_t.rearrange("p (c n) -> p c n", c=4),
                              wf_d[l].rearrange("c p n -> p c n"))
            if not fast:
                p12_t = sb.tile([R, 6 * D], f32, tag="p12", bufs=1,
                                name=f"p12_{l}")
                nc.sync.dma_start(p12_t.rearrange("p (g n) -> p g n", g=6),
                                  p12_d[l].rearrange("g p n -> p g n"))
                ln1w = p12_t[:, 0 * D: 1 * D]
                ln1b = p12_t[:, 1 * D: 2 * D]
                ln2w = p12_t[:, 2 * D: 3 * D]
                ln2b = p12_t[:, 3 * D: 4 * D]
                bo12 = p12_t[:, 4 * D: 5 * D]
                bf12 = p12_t[:, 5 * D: 6 * D]
            else:
                ln1w = ln1b = ln2w = ln2b = bo12 = bf12 = None

            def quake_rsqrt(var):
                # 1/sqrt(var) via bit-trick + 1 Newton iter (max err ~0.17%)
                yq = sb.tile([R, 1], f32, tag="lnyq", bufs=2)
                yi = yq.bitcast(i32)
                nc.vector.tensor_scalar(yi, var.bitcast(i32), 1, 0,
                                        OP.arith_shift_right, OP.bitwise_xor)
                nc.vector.tensor_tensor(out=yi, in0=qmag[:], in1=yi,
                                        op=OP.subtract)
                sq = sb.tile([R, 1], f32, tag="lnsq", bufs=2)
                nc.vector.tensor_tensor(out=sq[:], in0=yq[:], in1=yq[:],
                                        op=OP.mult)
                nc.vector.tensor_tensor(out=sq[:], in0=sq[:], in1=var,
                                        op=OP.mult)
                nc.vector.tensor_scalar(sq[:], sq[:], -0.5, 1.5,
                                        OP.mult, OP.add)
                nc.vector.tensor_tensor(out=yq[:], in0=yq[:], in1=sq[:],
                                        op=OP.mult)
                return yq

            def layer_norm(xin, s1, wln, bln):
                # mean from the fused residual row-sum; E[x^2] on Act
                # (Square shares the exp act table) in parallel with DVE
                mu = sb.tile([R, 1], f32, tag="lnmu", bufs=2)
                nc.vector.tensor_scalar_mul(mu[:], s1[:], 1.0 / D)
                sqs = sb.tile([R, D], f32, tag="scr", bufs=2)
                ss = sb.tile([R, 1], f32, tag="lnss", bufs=2)
                nc.scalar.activation(sqs[:], xin, AF.Square, accum_out=ss[:])
                mu2 = sb.tile([R, 1], f32, tag="lnmu2", bufs=2)
                nc.vector.tensor_tensor(out=mu2[:], in0=mu[:], in1=mu[:],
                                        op=OP.mult)
                var = sb.tile([R, 1], f32, tag="lnvar", bufs=2)
                nc.vector.tensor_scalar(var[:], ss[:], 1.0 / D, mu2[:],
                                        OP.mult, OP.subtract)
                yq = quake_rsqrt(var[:])
                hb = sb.tile([R, D], bf16, tag="lnhb", bufs=2)
                if fast:
                    nc.vector.tensor_scalar(hb[:], xin, mu[:], yq[:],
                                            OP.subtract, OP.mult)
                    return hb
                hh = sb.tile([R, D], f32, tag="lnh", bufs=2)
                nc.vector.tensor_scalar(hh[:], xin, mu[:], yq[:],
                                        OP.subtract, OP.mult)
                nc.vector.tensor_tensor(out=hh[:], in0=hh[:], in1=wln, op=OP.mult)
                nc.vector.tensor_tensor(out=hb[:], in0=hh[:], in1=bln, op=OP.add)
                return hb

            h1 = layer_norm(x[:], s1n, ln1w, ln1b)

            # ---- hT2all [128, 48] bf16: h^T, col = 12c + 3e + tq ----
            hT2all = sb.tile([128, 4 * R], bf16, tag="hT2", bufs=2,
                             name=f"hT2_{l}")
            tp1 = ps.tile([128, 4 * R], bf16, tag="sm", bufs=3)
            for c in range(NPAIR):
                nc.tensor.transpose(tp1[:, 12 * c: 12 * (c + 1)],
                                    h1[:, 128 * c: 128 * (c + 1)],
                                    i128[0:R, 0:R])
            nc.vector.tensor_copy(hT2all[:], tp1[:])

            # ---- QKV projections (one matmul each for Q^T, K^T) ----
            q2d = sb.tile([128, 512], f8, tag="q2d", bufs=2, name=f"q2d_{l}")
            nc.gpsimd.memset(q2d[:], 0.0)
            q2dv = q2d.rearrange("p (g e i m) -> p g e i m", g=2, e=BB, i=2)
            knT = sb.tile([128, 256], f8, tag="knT", bufs=2, name=f"knT_{l}")
            nc.gpsimd.memset(knT[:], 0.0)
            knv = knT.rearrange("p (e g i m) -> p e g i m", e=BB, g=2, i=2)

            qT2all = ps.tile([128, 4 * R], f32, tag="sm", bufs=3)
            nc.tensor.matmul(qT2all[:], wq2all[:, 128 * l: 128 * (l + 1)],
                             hT2all[:])
            qv = qT2all.rearrange("p (c e t) -> p c e t", c=4, e=BB)
            for c in range(NPAIR):
                g, i = c % 2, c // 2
                for hf in range(2):
                    nc.vector.tensor_copy(
                        q2dv[64 * hf: 64 * hf + 64, g, :, i,
                             6 * i + 3 * hf: 6 * i + 3 * hf + 3],
                        qv[64 * hf: 64 * hf + 64, c, :, :])
            kT2all = ps.tile([128, 4 * R], f32, tag="sm", bufs=3)
            nc.tensor.matmul(kT2all[:], wk2all[:, 128 * l: 128 * (l + 1)],
                             hT2all[:])
            kv = kT2all.rearrange("p (i g e t) -> p e g i t", i=2, g=2, e=BB)
            for i in range(2):
                nc.vector.tensor_copy(knv[:, :, :, i, 0:TN], kv[:, :, :, i, :])
            vn_ps = ps.tile([R, D], f32, tag="vn", bufs=1)
            for c in range(NPAIR):
                nc.tensor.matmul(vn_ps[:, 128 * c: 128 * (c + 1)],
                                 hT2all[:, 12 * c: 12 * (c + 1)],
                                 wv2all[:, 128 * l: 128 * (l + 1)])
            vnsb = sb.tile([R, D], f8, tag="vnsb", bufs=2)
            nc.scalar.copy(vnsb[:], vn_ps[:])
            Vn = [sb.tile([TN, D], f8, tag="Vn", bufs=8, name=f"Vn_{l}_{e}")
                  for e in range(BB)]
            for e in range(BB):
                nc.sync.dma_start(Vn[e][:], vnsb[3 * e: 3 * e + TN, :])

            # ---- new-token E, transposed: EnT[tn, 64e + 32g + m] ----
            EnT = ps.tile([16, 256], f32, tag="sm", bufs=3)
            nc.vector.memset(EnT[:], 0.0)
            for e in range(BB):
                for g in range(2):
                    nc.tensor.matmul(
                        EnT[0:16, 64 * e + 32 * g: 64 * e + 32 * g + 32],
                        knv[:, e, g, :, :], q2dv[:, g, e, :, :],
                        perf_mode=DR)
            nc.vector.tensor_tensor(out=EnT[:], in0=EnT[:], in1=negnT[:],
                                    op=OP.add)
            AnT = sb.tile([16, 256], f8, tag="AnT", bufs=2, name=f"AnT_{l}")
            nc.scalar.activation(AnT[:], EnT[:], AF.Exp, scale=ESC)

            # ---- attention per env, software-pipelined ----
            CTall = sb.tile([128, 64], f8, tag="CT", bufs=2, name=f"CT_{l}")
            nc.gpsimd.memset(CTall[:], 0.0)
            ctv = CTall.rearrange("p (g b m) -> p g b m", g=2, b=2)
            vFv = vFA.rearrange("p (e j g i b dd) -> p e j g i b dd",
                                e=BB, j=4, g=2, i=2, b=2)

            def emit_E(e):
                E = ps.tile([128, 512], f32, tag="eb", bufs=2,
                            name=f"E_{l}_{e}")
                for g in range(2):
                    for h2 in range(2):
                        rb = 32 * (2 * g + h2)
                        for i in range(2):
                            nc.tensor.matmul(
                                E[rb: rb + 32, 0:512],
                                q2dv[:, g, e, i, :],
                                ktA[:, 4096 * e + 1024 * (g + 2 * i) + 512 * h2:
                                    4096 * e + 1024 * (g + 2 * i) + 512 * (h2 + 1)],
                                start=(i == 0), stop=(i == 1),
                                tile_position=(0, rb))
                A4 = sb.tile([128, 512], bf16, tag="A4", bufs=2,
                             name=f"A4_{l}_{e}")
                acc = sb.tile([128, 1], f32, tag="acc", bufs=2)
                nc.scalar.activation(A4[:], E[:], AF.Exp, scale=ESC,
                                     accum_out=acc[:])
                # A^T: col = 128jj + 64g + 32i + m  (i = h2 = t-half)
                ATp = ps.tile([128, 512], bf16, tag="sm", bufs=3)
                for jj in range(4):
                    nc.tensor.transpose(ATp[:, 128 * jj: 128 * (jj + 1)],
                                        A4[:, 128 * jj: 128 * (jj + 1)],
                                        i128[:, :])
                AT = sb.tile([128, 512], f8, tag="AT", bufs=2,
                             name=f"AT_{l}_{e}")
                nc.scalar.copy(AT[:], ATp[:])
                ATv = AT.rearrange("p (j g i m) -> p j g i m", j=4, g=2, i=2)
                # denominators: fold h2 halves of acc (Act bias allows the
                # cross-partition-base read), add new-token sums, rcp on DVE
                dnew = ps.tile([32, 4], f32, tag="sm", bufs=3)
                for g in range(2):
                    nc.tensor.matmul(
                        dnew[0:32, 2 * g: 2 * g + 2],
                        AnT[0:TN, 64 * e + 32 * g: 64 * e + 32 * g + 32],
                        o2f8[0:TN, 0:2])
                ds = sb.tile([32, 2], f32, tag="ds", bufs=2)
                for g in range(2):
                    nc.scalar.activation(ds[:, g: g + 1],
                                         acc[64 * g: 64 * g + 32, :],
                                         AF.Identity,
                                         bias=acc[64 * g + 32: 64 * g + 64, :])
                rcp = sb.tile([32, 2], f32, tag="rcp", bufs=2)
                nc.vector.tensor_tensor(
                    out=rcp[:], in0=ds[:],
                    in1=dnew.rearrange("p (g o) -> p g o", g=2)[:, :, 0],
                    op=OP.add)
                nc.vector.tensor_tensor(out=rcp[:], in0=rcp[:],
                                        in1=npad4[:, 2 * e: 2 * e + 2],
                                        op=OP.subtract)
                nc.vector.reciprocal(rcp[:], rcp[:])
                return ATv, rcp

            def emit_attn(e, ATv, rcp):
                otp = ps.tile([128, 48], f32, tag="sm", bufs=3)
                for g in range(2):
                    og = ps.tile([32, 256], f32, tag="og", bufs=2,
                                 name=f"og_{l}_{e}_{g}")
                    for jp in range(4):
                        nc.tensor.matmul(
                            og[0:32, 0:256],
                            ATv[:, jp, g, :, :],
                            vFv[:, e, jp, g, :, :, :],
                            perf_mode=DR, start=(jp == 0), stop=False)
                    nc.tensor.matmul(
                        og[0:32, 0:256],
                        AnT[0:TN, 64 * e + 32 * g: 64 * e + 32 * g + 32],
                        Vn[e].rearrange("p (b d2) -> p b d2", b=2)[
                            :, :, 128 * g: 128 * (g + 1)],
                        start=False, stop=True)
                    onrm = sb.tile([32, 256], bf16, tag="onrm", bufs=4)
                    nc.vector.tensor_scalar_mul(onrm[:], og[:],
                                                rcp[:, g: g + 1])
                    # O^T gather: identity-slice matmuls select the 6 valid
                    # rows per (g, b); otp col = 12c + 3hf + tq
                    for b in range(2):
                        c = g + 2 * b
                        nc.tensor.matmul(
                            otp[:, 12 * c: 12 * c + 6],
                            onrm[:, 128 * b: 128 * (b + 1)],
                            i128[0:32, 6 * b: 6 * b + 6])
                src = otp.rearrange("p (b g u) -> p g b u", b=2, g=2)
                for hf in range(2):
                    nc.vector.tensor_copy(
                        ctv[64 * hf: 64 * hf + 64, :, :, 3 * e: 3 * e + 3],
                        src[64 * hf: 64 * hf + 64, :, :,
                            3 * hf: 3 * hf + 3])

            pend = emit_E(0)
            for e in range(BB):
                nxt = emit_E(e + 1) if e + 1 < BB else None
                emit_attn(e, *pend)
                pend = nxt

            # ---- output projection (fp8 DR over b-pairs) + residual ----
            xo = ps.tile([16, D], f32, tag="sm", bufs=3)
            wov = wo_t.rearrange("p (g b n) -> p g b n", g=2, b=2)
            for nh in range(2):
                for g in range(2):
                    nc.tensor.matmul(
                        xo[0:16, 256 * nh: 256 * (nh + 1)],
                        ctv[:, g, :, :],
                        wov[:, g, :, 256 * nh: 256 * (nh + 1)],
                        perf_mode=DR, start=(g == 0), stop=(g == 1))
            s1a = sb.tile([R, 1], f32, tag="s1", bufs=4)
            if fast:
                nc.vector.scalar_tensor_tensor(
                    out=x[:], in0=xo[0:R, :], scalar=0.0, in1=x[:],
                    op0=OP.add, op1=OP.add, accum_out=s1a[:])
            else:
                xt = sb.tile([R, D], f32, tag="scr", bufs=2)
                nc.vector.tensor_tensor(out=xt[:], in0=xo[0:R, :], in1=bo12,
                                        op=OP.add)
                nc.vector.scalar_tensor_tensor(
                    out=x[:], in0=xt[:], scalar=0.0, in1=x[:],
                    op0=OP.add, op1=OP.add, accum_out=s1a[:])

            # ---- FFN ----
            h2t = layer_norm(x[:], s1a, ln2w, ln2b)
            HTall = sb.tile([128, 4 * R], bf16, tag="HT", bufs=2,
                            name=f"HT_{l}")
            tp2 = ps.tile([128, 4 * R], bf16, tag="sm", bufs=3)
            for c in range(NPAIR):
                nc.tensor.transpose(tp2[:, 12 * c: 12 * (c + 1)],
                                    h2t[:, 128 * c: 128 * (c + 1)],
                                    i128[0:R, 0:R])
            nc.vector.tensor_copy(HTall[:], tp2[:])
            ff = ps.tile([R, D], f32, tag="sm", bufs=3)
            for c in range(NPAIR):
                nc.tensor.matmul(ff[:], HTall[:, 12 * c: 12 * (c + 1)],
                                 wf_t[:, c * D: (c + 1) * D],
                                 start=(c == 0), stop=(c == NPAIR - 1))
            s1n = sb.tile([R, 1], f32, tag="s1", bufs=4)
            if fast:
                nc.vector.scalar_tensor_tensor(
                    out=x[:], in0=ff[:], scalar=0.0, in1=x[:],
                    op0=OP.max, op1=OP.add, accum_out=s1n[:])
            else:
                ft = sb.tile([R, D], f32, tag="scr", bufs=2)
                nc.vector.tensor_tensor(out=ft[:], in0=ff[:], in1=bf12, op=OP.add)
                nc.scalar.activation(ft[:], ft[:], AF.Relu)
                nc.vector.scalar_tensor_tensor(
                    out=x[:], in0=ft[:], scalar=0.0, in1=x[:],
                    op0=OP.add, op1=OP.add, accum_out=s1n[:])

        nc.sync.dma_start(out_d[:], x[:])

    nc.compile()
    return nc


def _prep_inputs(x, past_k, past_v, pad_mask, ln1_w, ln1_b, ln2_w, ln2_b,
                 Wq, Wk, Wv, Wo, bo, Wf, bf):
    import ml_dtypes
    f = np.float32
    b16 = ml_dtypes.bfloat16
    fp8 = ml_dtypes.float8_e4m3
    x = np.ascontiguousarray(x, f)
    past_k = np.asarray(past_k, f)
    past_v = np.asarray(past_v, f)
    pad_mask = np.asarray(pad_mask)

    def blk2(wT):
        out = np.zeros((L, 128, 128), f)
        out[:, 0:64, 0:64] = wT
        out[:, 64:128, 64:128] = wT
        return out.astype(b16)
    wq2 = blk2(np.transpose(np.asarray(Wq, f), (0, 2, 1)))
    wk2 = blk2(np.transpose(np.asarray(Wk, f), (0, 2, 1)))
    wv2 = blk2(np.transpose(np.asarray(Wv, f), (0, 2, 1)))
    woT = np.transpose(np.asarray(Wo, f), (0, 2, 1)).reshape(L, 2, 2, 128, D)
    # wo2[l, p, 1024g + 512b + n] = Wo^T[l, 128(g+2b) + p, n]; c = 2b + g in
    # woT's reshape is (b, g) so transpose to (l, p, g, b, n)
    wo2 = np.transpose(woT, (0, 3, 2, 1, 4)).reshape(L, 128, 2048)
    wo2 = np.clip(wo2, -240.0, 240.0).astype(fp8)
    wfT = np.transpose(np.asarray(Wf, f), (0, 2, 1)).reshape(L, 4, 128, D).astype(b16)
    p12 = np.stack(
        [np.broadcast_to(np.asarray(a, f)[:, None, :], (L, R, D))
         for a in (ln1_w, ln1_b, ln2_w, ln2_b, bo, bf)], axis=1)
    p12 = np.ascontiguousarray(p12)
    i128 = np.eye(128, dtype=b16)
    ones2 = np.ones((128, 32), f).astype(fp8)

    def to8(a):
        return np.clip(a, -240.0, 240.0).astype(fp8)

    in_maps = []
    for cc in range(NC):
        bs = slice(cc * BB, (cc + 1) * BB)
        pk = past_k[:, bs]                      # (L, BB, H, TP, Dh)
        pv = past_v[:, bs]
        # ktT[l, e, 64hf+d, 1024pr+t] = pk[l, e, 2pr+hf, t, d]
        kt = pk.reshape(L, BB, NPAIR, 2, TP, Dh)
        kt = np.transpose(kt, (0, 1, 3, 5, 2, 4))    # l, e, hf, d, pr, t
        kt = to8(np.ascontiguousarray(kt.reshape(L, BB, 128, NPAIR * TP)))
        # vF[l, e, p, 1024jp+512g+256i+128b+64hf+d] =
        #    pv[l, e, 4b+2g+hf, 512i+128jp+p, d]
        vf = pv.reshape(L, BB, 2, 2, 2, 2, NJ // 2, 128, Dh)
        #                      b  g  hf i  jp      p    d
        vf = np.transpose(vf, (0, 1, 7, 6, 3, 5, 2, 4, 8))
        #   -> l, e, p, jp, g, i, b, hf, d
        vf = to8(np.ascontiguousarray(vf.reshape(L, BB, 128, NPAIR * TP)))

        pm = np.asarray(pad_mask[bs])           # (BB, Tt) bool
        npad_e = (TP - pm[:, :TP].sum(axis=1)).astype(f)   # (BB,)

        npad4 = np.zeros((32, 2 * BB), f)
        for e in range(BB):
            npad4[0:12, 2 * e: 2 * e + 2] = npad_e[e]
        negnT = np.full((16, 256), NEG, f)
        for e in range(BB):
            for tn in range(TN):
                for g in range(2):
                    for m in range(12):
                        tq = m % 3
                        if tn <= tq and bool(pm[e, TP + tn]):
                            negnT[tn, 64 * e + 32 * g + m] = 0.0

        in_maps.append({
            "x0": np.ascontiguousarray(x[bs].reshape(R, D)),
            "ktT": kt, "vF": vf,
            "wq2": wq2, "wk2": wk2, "wv2": wv2,
            "wo2": wo2, "wfT": wfT, "p12": p12,
            "i128b": i128, "ones2f8": ones2,
            "negnT": negnT, "npad4": npad4,
        })
    return in_maps


_CACHE = {}


def kernel(**inputs):
    import os
    import sys
    for p in ("/opt/trn_rl_repo", "/opt/pypackages"):
        if p not in sys.path:
            sys.path.insert(0, p)
    os.environ.setdefault("JAX_PLATFORMS", "")
    from concourse.bass_utils import run_bass_kernel_spmd

    in_maps = _prep_inputs(**inputs)
    fast = all(np.allclose(np.asarray(inputs[k]), 1.0) for k in ("ln1_w", "ln2_w")) \
        and all(np.allclose(np.asarray(inputs[k]), 0.0)
                for k in ("ln1_b", "ln2_b", "bo", "bf"))
    key = f"nc_{fast}"
    if key not in _CACHE:
        _CACHE[key] = _build_bass(fast)
    nc = _CACHE[key]
    res = run_bass_kernel_spmd(nc, in_maps, core_ids=list(range(NC)))
    out = np.concatenate([r["xout"].reshape(BB, TN, D) for r in res.results], axis=0)
    return out.astype(np.float32)


# revision 28
# speedup vs baseline: 1.4225x; 1.0000x over previous
"""Trainium2 Bass kernel: 4-layer decode-attention transformer block (fp8 KV).

Shapes (hardcoded): L=4, B=32, H=8, Dh=64, D=512, TP=1024, TN=3, Tt=1027.
Sharding: data-parallel over B across 8 cores (4 envs each); params replicated.

v5 design notes (evolved from the 234.6us bf16 v2 baseline; v4 fp8 measured
210.9us, PE-bound on instruction count):
 - K/V/Wo streamed from HBM in fp8 e4m3 (~20.5MB/core vs 38.8MB).
 - E packed [128, 512] per env: row = 32*(2g + h2) + m, col = key t within
   the h2 half (m = 6i + 3hf + tq labels the two pair-blocks g+2i). This
   packing lets ONE Act exp call (512 cols) cover the whole env, with
   accum_out producing the softmax denominators for free.
 - QK^T is 8 plain fp8 matmuls [32,512] per env (DoubleRow outputs must
   start at partition 0 - walrus codegen constraint - so DR can't write the
   row-packed E); AV, denominator-fold tails, new-token E, and the Wo
   projection DO use fp8 DoubleRow with base-0 outputs.
 - A^T via 4 PE transposes [128,128] -> one scalar.copy; the AT column
   layout 128*jj + 64g + 32i + m falls out with i = h2 as the DoubleRow
   k-tile pair (t, t+512), matching the vF host layout.
 - Denominator: exp accum_out [128,1] folded pairwise by 2 Act Identity ops
   with cross-partition-base bias (engines allow that only via Act bias);
   plus tiny new-token ones-matmuls; rcp on DVE.
 - O gather: plain matmuls with identity slices select the 6 valid rows per
   (g,b) out of onrm^T, so the CT gather is 2 strided copies (on Pool).
 - Pool engine (nc.gpsimd) offloads residual adds, copies, quake rsqrt.
 - LN: DVE bn_stats/bn_aggr; 1/sqrt(var) via quake bit-trick + 2 Newton
   iterations on Pool (keeps Act tables pinned to the exp set).
"""

import numpy as np

L, B, H, Dh, D, TP, TN = 4, 32, 8, 64, 512, 1024, 3
Tt = TP + TN
NC = 8
BB = B // NC          # envs per core = 4
R = BB * TN           # x rows per core = 12
NJ = TP // 128        # t-chunks of 128 = 8
NPAIR = H // 2        # head pairs = 4
EPS = 1e-5
NEG = -1e9
ESC = 0.125           # 1/sqrt(Dh), applied as Act scale at exp time
QMAGIC = 1597463007.0  # 0x5f3759df


def _build_bass(fast=True):
    import concourse.bass as bass
    import concourse.mybir as mybir
    import concourse.tile as tile
    from concourse import bacc

    f32 = mybir.dt.float32
    i32 = mybir.dt.int32
    bf16 = mybir.dt.bfloat16
    f8 = mybir.dt.float8e4
    AF = mybir.ActivationFunctionType
    OP = mybir.AluOpType
    DR = mybir.MatmulPerfMode.DoubleRow
    AX = mybir.AxisListType

    nc = bacc.Bacc("TRN2", target_bir_lowering=False, debug=False, num_devices=NC)

    x_d = nc.dram_tensor("x0", [R, D], f32, kind="ExternalInput")
    # K^T per (l, env): rows 64*hf+d, cols 1024*pr + t
    kt_d = nc.dram_tensor("ktT", [L, BB, 128, NPAIR * TP], f8, kind="ExternalInput")
    # V per (l, env): rows p, cols 1024*jp + 512*g + 256*i + 128*b + 64*hf + d
    #  = V[head 4b+2g+hf, t = 512*i + 128*jp + p, d]
    vf_d = nc.dram_tensor("vF", [L, BB, 128, NPAIR * TP], f8, kind="ExternalInput")
    wq_d = nc.dram_tensor("wq2", [L, 128, 128], bf16, kind="ExternalInput")
    wk_d = nc.dram_tensor("wk2", [L, 128, 128], bf16, kind="ExternalInput")
    wv_d = nc.dram_tensor("wv2", [L, 128, 128], bf16, kind="ExternalInput")
    # Wo as fp8 DR pairs: col = 1024g + 512b + n  (c = g + 2b)
    wo_d = nc.dram_tensor("wo2", [L, 128, 2 * D * 2], f8, kind="ExternalInput")
    wf_d = nc.dram_tensor("wfT", [L, 4, 128, D], bf16, kind="ExternalInput")
    p12_d = nc.dram_tensor("p12", [L, 6, R, D], f32, kind="ExternalInput")
    i128_d = nc.dram_tensor("i128b", [128, 128], bf16, kind="ExternalInput")
    ones_d = nc.dram_tensor("ones2f8", [128, 32], f8, kind="ExternalInput")
    # new-token causal/pad bias, rows tn, cols 64e + 32g + m
    negnT_d = nc.dram_tensor("negnT", [16, 256], f32, kind="ExternalInput")
    # padded-slot count, rows m, col 2e + g
    npad_d = nc.dram_tensor("npad4", [32, 2 * BB], f32, kind="ExternalInput")
    out_d = nc.dram_tensor("xout", [R, D], f32, kind="ExternalOutput")

    from contextlib import ExitStack

    with tile.TileContext(nc) as tc, ExitStack() as st:
        consts = st.enter_context(tc.tile_pool(name="consts", bufs=1))
        sb = st.enter_context(tc.tile_pool(name="sb", bufs=1))
        ps = st.enter_context(tc.tile_pool(name="ps", bufs=1, space="PSUM"))

        x = consts.tile([R, D], f32)
        nc.sync.dma_start(x[:], x_d[:])
        # layer-0 K/V first on the Sync queue so env-0's keys arrive before
        # the first E matmuls instead of behind all the consts transfers
        ktA0 = sb.tile([128, BB * NPAIR * TP], f8, tag="ktA", bufs=2,
                       name="ktA_0")
        for e in range(BB):
            nc.sync.dma_start(ktA0[:, 4096 * e: 4096 * (e + 1)],
                              kt_d[0, e])
        vFA0 = sb.tile([128, BB * NPAIR * TP], f8, tag="vFA", bufs=2,
                       name="vFA_0")
        nc.sync.dma_start(vFA0.rearrange("p (e n) -> p e n", e=BB),
                          vf_d[0].rearrange("e p n -> p e n"))
        i128 = consts.tile([128, 128], bf16)
        nc.sync.dma_start(i128[:], i128_d[:])
        o2f8 = consts.tile([128, 32], f8)
        nc.sync.dma_start(o2f8[:], ones_d[:])
        negnT = consts.tile([16, 256], f32)
        nc.sync.dma_start(negnT[:], negnT_d[:])
        npad4 = consts.tile([32, 2 * BB], f32)
        nc.sync.dma_start(npad4[:], npad_d[:])
        qmag = consts.tile([R, 1], i32)
        nc.vector.memset(qmag[:], QMAGIC)

        s1n = consts.tile([R, 1], f32)
        nc.vector.tensor_reduce(s1n[:], x[:], AX.X, OP.add)

        wq2all = consts.tile([128, L * 128], bf16)
        nc.sync.dma_start(wq2all.rearrange("p (l n) -> p l n", l=L),
                          wq_d.rearrange("l p n -> p l n"))
        wk2all = consts.tile([128, L * 128], bf16)
        nc.sync.dma_start(wk2all.rearrange("p (l n) -> p l n", l=L),
                          wk_d.rearrange("l p n -> p l n"))
        wv2all = consts.tile([128, L * 128], bf16)
        nc.sync.dma_start(wv2all.rearrange("p (l n) -> p l n", l=L),
                          wv_d.rearrange("l p n -> p l n"))

        for l in range(L):
            # ---- per-layer loads ----
            if l == 0:
                ktA, vFA = ktA0, vFA0
            else:
                ktA = sb.tile([128, BB * NPAIR * TP], f8, tag="ktA", bufs=2,
                              name=f"ktA_{l}")
                for e in range(BB):
                    nc.sync.dma_start(ktA[:, 4096 * e: 4096 * (e + 1)],
                                      kt_d[l, e])
                vFA = sb.tile([128, BB * NPAIR * TP], f8, tag="vFA", bufs=2,
                              name=f"vFA_{l}")
                nc.sync.dma_start(vFA.rearrange("p (e n) -> p e n", e=BB),
                                  vf_d[l].rearrange("e p n -> p e n"))
            wo_t = sb.tile([128, 4 * D], f8, tag="wo", bufs=2, name=f"wo_{l}")
            nc.sync.dma_start(wo_t[:], wo_d[l])
            wf_t = sb.tile([128, 4 * D], bf16, tag="wf", bufs=2, name=f"wf_{l}")
            nc.sync.dma_start(wf_t.rearrange("p (c n) -> p c n", c=4),
                              wf_d[l].rearrange("c p n -> p c n"))
            if not fast:
                p12_t = sb.tile([R, 6 * D], f32, tag="p12", bufs=1,
                                name=f"p12_{l}")
                nc.sync.dma_start(p12_t.rearrange("p (g n) -> p g n", g=6),
                                  p12_d[l].rearrange("g p n -> p g n"))
                ln1w = p12_t[:, 0 * D: 1 * D]
                ln1b = p12_t[:, 1 * D: 2 * D]
                ln2w = p12_t[:, 2 * D: 3 * D]
                ln2b = p12_t[:, 3 * D: 4 * D]
                bo12 = p12_t[:, 4 * D: 5 * D]
                bf12 = p12_t[:, 5 * D: 6 * D]
            else:
                ln1w = ln1b = ln2w = ln2b = bo12 = bf12 = None

            def quake_rsqrt(var):
                # 1/sqrt(var) via bit-trick + 1 Newton iter (max err ~0.17%)
                yq = sb.tile([R, 1], f32, tag="lnyq", bufs=2)
                yi = yq.bitcast(i32)
                nc.vector.tensor_scalar(yi, var.bitcast(i32), 1, 0,
                                        OP.arith_shift_right, OP.bitwise_xor)
                nc.vector.tensor_tensor(out=yi, in0=qmag[:], in1=yi,
                                        op=OP.subtract)
                sq = sb.tile([R, 1], f32, tag="lnsq", bufs=2)
                nc.vector.tensor_tensor(out=sq[:], in0=yq[:], in1=yq[:],
                                        op=OP.mult)
                nc.vector.tensor_tensor(out=sq[:], in0=sq[:], in1=var,
                                        op=OP.mult)
                nc.vector.tensor_scalar(sq[:], sq[:], -0.5, 1.5,
                                        OP.mult, OP.add)
                nc.vector.tensor_tensor(out=yq[:], in0=yq[:], in1=sq[:],
                                        op=OP.mult)
                return yq

            def layer_norm(xin, s1, wln, bln):
                # mean from the fused residual row-sum; E[x^2] on Act
                # (Square shares the exp act table) in parallel with DVE
                mu = sb.tile([R, 1], f32, tag="lnmu", bufs=2)
                nc.vector.tensor_scalar_mul(mu[:], s1[:], 1.0 / D)
                sqs = sb.tile([R, D], f32, tag="scr", bufs=2)
                ss = sb.tile([R, 1], f32, tag="lnss", bufs=2)
                nc.scalar.activation(sqs[:], xin, AF.Square, accum_out=ss[:])
                mu2 = sb.tile([R, 1], f32, tag="lnmu2", bufs=2)
                nc.vector.tensor_tensor(out=mu2[:], in0=mu[:], in1=mu[:],
                                        op=OP.mult)
                var = sb.tile([R, 1], f32, tag="lnvar", bufs=2)
                nc.vector.tensor_scalar(var[:], ss[:], 1.0 / D, mu2[:],
                                        OP.mult, OP.subtract)
                yq = quake_rsqrt(var[:])
                hb = sb.tile([R, D], bf16, tag="lnhb", bufs=2)
                if fast:
                    nc.vector.tensor_scalar(hb[:], xin, mu[:], yq[:],
                                            OP.subtract, OP.mult)
                    return hb
                hh = sb.tile([R, D], f32, tag="lnh", bufs=2)
                nc.vector.tensor_scalar(hh[:], xin, mu[:], yq[:],
                                        OP.subtract, OP.mult)
                nc.vector.tensor_tensor(out=hh[:], in0=hh[:], in1=wln, op=OP.mult)
                nc.vector.tensor_tensor(out=hb[:], in0=hh[:], in1=bln, op=OP.add)
                return hb

            h1 = layer_norm(x[:], s1n, ln1w, ln1b)

            # ---- hT2all [128, 48] bf16: h^T, col = 12c + 3e + tq ----
            hT2all = sb.tile([128, 4 * R], bf16, tag="hT2", bufs=2,
                             name=f"hT2_{l}")
            tp1 = ps.tile([128, 4 * R], bf16, tag="sm", bufs=3)
            for c in range(NPAIR):
                nc.tensor.transpose(tp1[:, 12 * c: 12 * (c + 1)],
                                    h1[:, 128 * c: 128 * (c + 1)],
                                    i128[0:R, 0:R])
            nc.vector.tensor_copy(hT2all[:], tp1[:])

            # ---- QKV projections (one matmul each for Q^T, K^T) ----
            q2d = sb.tile([128, 512], f8, tag="q2d", bufs=2, name=f"q2d_{l}")
            nc.gpsimd.memset(q2d[:], 0.0)
            q2dv = q2d.rearrange("p (g e i m) -> p g e i m", g=2, e=BB, i=2)
            knT = sb.tile([128, 256], f8, tag="knT", bufs=2, name=f"knT_{l}")
            nc.gpsimd.memset(knT[:], 0.0)
            knv = knT.rearrange("p (e g i m) -> p e g i m", e=BB, g=2, i=2)

            qT2all = ps.tile([128, 4 * R], f32, tag="sm", bufs=3)
            nc.tensor.matmul(qT2all[:], wq2all[:, 128 * l: 128 * (l + 1)],
                             hT2all[:])
            qv = qT2all.rearrange("p (c e t) -> p c e t", c=4, e=BB)
            for c in range(NPAIR):
                g, i = c % 2, c // 2
                for hf in range(2):
                    nc.vector.tensor_copy(
                        q2dv[64 * hf: 64 * hf + 64, g, :, i,
                             6 * i + 3 * hf: 6 * i + 3 * hf + 3],
                        qv[64 * hf: 64 * hf + 64, c, :, :])
            kT2all = ps.tile([128, 4 * R], f32, tag="sm", bufs=3)
            nc.tensor.matmul(kT2all[:], wk2all[:, 128 * l: 128 * (l + 1)],
                             hT2all[:])
            kv = kT2all.rearrange("p (i g e t) -> p e g i t", i=2, g=2, e=BB)
            for i in range(2):
                nc.vector.tensor_copy(knv[:, :, :, i, 0:TN], kv[:, :, :, i, :])
            vn_ps = ps.tile([R, D], f32, tag="vn", bufs=1)
            for c in range(NPAIR):
                nc.tensor.matmul(vn_ps[:, 128 * c: 128 * (c + 1)],
                                 hT2all[:, 12 * c: 12 * (c + 1)],
                                 wv2all[:, 128 * l: 128 * (l + 1)])
            vnsb = sb.tile([R, D], f8, tag="vnsb", bufs=2)
            nc.scalar.copy(vnsb[:], vn_ps[:])
            Vn = [sb.tile([TN, D], f8, tag="Vn", bufs=8, name=f"Vn_{l}_{e}")
                  for e in range(BB)]
            for e in range(BB):
                nc.sync.dma_start(Vn[e][:], vnsb[3 * e: 3 * e + TN, :])

            # ---- new-token E, transposed: EnT[tn, 64e + 32g + m] ----
            EnT = ps.tile([16, 256], f32, tag="sm", bufs=3)
            nc.vector.memset(EnT[:], 0.0)
            for e in range(BB):
                for g in range(2):
                    nc.tensor.matmul(
                        EnT[0:16, 64 * e + 32 * g: 64 * e + 32 * g + 32],
                        knv[:, e, g, :, :], q2dv[:, g, e, :, :],
                        perf_mode=DR)
            nc.vector.tensor_tensor(out=EnT[:], in0=EnT[:], in1=negnT[:],
                                    op=OP.add)
            AnT = sb.tile([16, 256], f8, tag="AnT", bufs=2, name=f"AnT_{l}")
            nc.scalar.activation(AnT[:], EnT[:], AF.Exp, scale=ESC)

            # ---- attention per env, software-pipelined ----
            CTall = sb.tile([128, 64], f8, tag="CT", bufs=2, name=f"CT_{l}")
            nc.gpsimd.memset(CTall[:], 0.0)
            ctv = CTall.rearrange("p (g b m) -> p g b m", g=2, b=2)
            vFv = vFA.rearrange("p (e j g i b dd) -> p e j g i b dd",
                                e=BB, j=4, g=2, i=2, b=2)

            def emit_E(e):
                E = ps.tile([128, 512], f32, tag="eb", bufs=2,
                            name=f"E_{l}_{e}")
                for g in range(2):
                    for h2 in range(2):
                        rb = 32 * (2 * g + h2)
                        for i in range(2):
                            nc.tensor.matmul(
                                E[rb: rb + 32, 0:512],
                                q2dv[:, g, e, i, :],
                                ktA[:, 4096 * e + 1024 * (g + 2 * i) + 512 * h2:
                                    4096 * e + 1024 * (g + 2 * i) + 512 * (h2 + 1)],
                                start=(i == 0), stop=(i == 1),
                                tile_position=(0, rb))
                A4 = sb.tile([128, 512], bf16, tag="A4", bufs=2,
                             name=f"A4_{l}_{e}")
                acc = sb.tile([128, 1], f32, tag="acc", bufs=2)
                nc.scalar.activation(A4[:], E[:], AF.Exp, scale=ESC,
                                     accum_out=acc[:])
                # A^T: col = 128jj + 64g + 32i + m  (i = h2 = t-half)
                ATp = ps.tile([128, 512], bf16, tag="sm", bufs=3)
                for jj in range(4):
                    nc.tensor.transpose(ATp[:, 128 * jj: 128 * (jj + 1)],
                                        A4[:, 128 * jj: 128 * (jj + 1)],
                                        i128[:, :])
                AT = sb.tile([128, 512], f8, tag="AT", bufs=2,
                             name=f"AT_{l}_{e}")
                nc.scalar.copy(AT[:], ATp[:])
                ATv = AT.rearrange("p (j g i m) -> p j g i m", j=4, g=2, i=2)
                # denominators: fold h2 halves of acc (Act bias allows the
                # cross-partition-base read), add new-token sums, rcp on DVE
                dnew = ps.tile([32, 4], f32, tag="sm", bufs=3)
                for g in range(2):
                    nc.tensor.matmul(
                        dnew[0:32, 2 * g: 2 * g + 2],
                        AnT[0:TN, 64 * e + 32 * g: 64 * e + 32 * g + 32],
                        o2f8[0:TN, 0:2])
                ds = sb.tile([32, 2], f32, tag="ds", bufs=2)
                for g in range(2):
                    nc.scalar.activation(ds[:, g: g + 1],
                                         acc[64 * g: 64 * g + 32, :],
                                         AF.Identity,
                                         bias=acc[64 * g + 32: 64 * g + 64, :])
                rcp = sb.tile([32, 2], f32, tag="rcp", bufs=2)
                nc.vector.tensor_tensor(
                    out=rcp[:], in0=ds[:],
                    in1=dnew.rearrange("p (g o) -> p g o", g=2)[:, :, 0],
                    op=OP.add)
                nc.vector.tensor_tensor(out=rcp[:], in0=rcp[:],
                                        in1=npad4[:, 2 * e: 2 * e + 2],
                                        op=OP.subtract)
                nc.vector.reciprocal(rcp[:], rcp[:])
                return ATv, rcp

            def emit_attn(e, ATv, rcp):
                otp = ps.tile([128, 48], f32, tag="sm", bufs=3)
                for g in range(2):
                    og = ps.tile([32, 256], f32, tag="og", bufs=2,
                                 name=f"og_{l}_{e}_{g}")
                    for jp in range(4):
                        nc.tensor.matmul(
                            og[0:32, 0:256],
                            ATv[:, jp, g, :, :],
                            vFv[:, e, jp, g, :, :, :],
                            perf_mode=DR, start=(jp == 0), stop=False)
                    nc.tensor.matmul(
                        og[0:32, 0:256],
                        AnT[0:TN, 64 * e + 32 * g: 64 * e + 32 * g + 32],
                        Vn[e].rearrange("p (b d2) -> p b d2", b=2)[
                            :, :, 128 * g: 128 * (g + 1)],
                        start=False, stop=True)
                    onrm = sb.tile([32, 256], bf16, tag="onrm", bufs=4)
                    nc.vector.tensor_scalar_mul(onrm[:], og[:],
                                                rcp[:, g: g + 1])
                    # O^T gather: identity-slice matmuls select the 6 valid
                    # rows per (g, b); otp col = 12c + 3hf + tq
                    for b in range(2):
                        c = g + 2 * b
                        nc.tensor.matmul(
                            otp[:, 12 * c: 12 * c + 6],
                            onrm[:, 128 * b: 128 * (b + 1)],
                            i128[0:32, 6 * b: 6 * b + 6])
                src = otp.rearrange("p (b g u) -> p g b u", b=2, g=2)
                for hf in range(2):
                    nc.vector.tensor_copy(
                        ctv[64 * hf: 64 * hf + 64, :, :, 3 * e: 3 * e + 3],
                        src[64 * hf: 64 * hf + 64, :, :,
                            3 * hf: 3 * hf + 3])

            pend = emit_E(0)
            for e in range(BB):
                nxt = emit_E(e + 1) if e + 1 < BB else None
                emit_attn(e, *pend)
                pend = nxt

            # ---- output projection (fp8 DR over b-pairs) + residual ----
            xo = ps.tile([16, D], f32, tag="sm", bufs=3)
            wov = wo_t.rearrange("p (g b n) -> p g b n", g=2, b=2)
            for nh in range(2):
                for g in range(2):
                    nc.tensor.matmul(
                        xo[0:16, 256 * nh: 256 * (nh + 1)],
                        ctv[:, g, :, :],
                        wov[:, g, :, 256 * nh: 256 * (nh + 1)],
                        perf_mode=DR, start=(g == 0), stop=(g == 1))
            s1a = sb.tile([R, 1], f32, tag="s1", bufs=4)
            if fast:
                nc.vector.scalar_tensor_tensor(
                    out=x[:], in0=xo[0:R, :], scalar=0.0, in1=x[:],
                    op0=OP.add, op1=OP.add, accum_out=s1a[:])
            else:
                xt = sb.tile([R, D], f32, tag="scr", bufs=2)
                nc.vector.tensor_tensor(out=xt[:], in0=xo[0:R, :], in1=bo12,
                                        op=OP.add)
                nc.vector.scalar_tensor_tensor(
                    out=x[:], in0=xt[:], scalar=0.0, in1=x[:],
                    op0=OP.add, op1=OP.add, accum_out=s1a[:])

            # ---- FFN ----
            h2t = layer_norm(x[:], s1a, ln2w, ln2b)
            HTall = sb.tile([128, 4 * R], bf16, tag="HT", bufs=2,
                            name=f"HT_{l}")
            tp2 = ps.tile([128, 4 * R], bf16, tag="sm", bufs=3)
            for c in range(NPAIR):
                nc.tensor.transpose(tp2[:, 12 * c: 12 * (c + 1)],
                                    h2t[:, 128 * c: 128 * (c + 1)],
                                    i128[0:R, 0:R])
            nc.vector.tensor_copy(HTall[:], tp2[:])
            ff = ps.tile([R, D], f32, tag="sm", bufs=3)
            for c in range(NPAIR):
                nc.tensor.matmul(ff[:], HTall[:, 12 * c: 12 * (c + 1)],
                                 wf_t[:, c * D: (c + 1) * D],
                                 start=(c == 0), stop=(c == NPAIR - 1))
            s1n = sb.tile([R, 1], f32, tag="s1", bufs=4)
            if fast:
                nc.vector.scalar_tensor_tensor(
                    out=x[:], in0=ff[:], scalar=0.0, in1=x[:],
                    op0=OP.max, op1=OP.add, accum_out=s1n[:])
            else:
                ft = sb.tile([R, D], f32, tag="scr", bufs=2)
                nc.vector.tensor_tensor(out=ft[:], in0=ff[:], in1=bf12, op=OP.add)
                nc.scalar.activation(ft[:], ft[:], AF.Relu)
                nc.vector.scalar_tensor_tensor(
                    out=x[:], in0=ft[:], scalar=0.0, in1=x[:],
                    op0=OP.add, op1=OP.add, accum_out=s1n[:])

        nc.sync.dma_start(out_d[:], x[:])

    nc.compile()
    return nc


def _prep_inputs(x, past_k, past_v, pad_mask, ln1_w, ln1_b, ln2_w, ln2_b,
                 Wq, Wk, Wv, Wo, bo, Wf, bf):
    import ml_dtypes
    f = np.float32
    b16 = ml_dtypes.bfloat16
    fp8 = ml_dtypes.float8_e4m3
    x = np.ascontiguousarray(x, f)
    past_k = np.asarray(past_k, f)
    past_v = np.asarray(past_v, f)
    pad_mask = np.asarray(pad_mask)

    def blk2(wT):
        out = np.zeros((L, 128, 128), f)
        out[:, 0:64, 0:64] = wT
        out[:, 64:128, 64:128] = wT
        return out.astype(b16)
    wq2 = blk2(np.transpose(np.asarray(Wq, f), (0, 2, 1)))
    wk2 = blk2(np.transpose(np.asarray(Wk, f), (0, 2, 1)))
    wv2 = blk2(np.transpose(np.asarray(Wv, f), (0, 2, 1)))
    woT = np.transpose(np.asarray(Wo, f), (0, 2, 1)).reshape(L, 2, 2, 128, D)
    # wo2[l, p, 1024g + 512b + n] = Wo^T[l, 128(g+2b) + p, n]; c = 2b + g in
    # woT's reshape is (b, g) so transpose to (l, p, g, b, n)
    wo2 = np.transpose(woT, (0, 3, 2, 1, 4)).reshape(L, 128, 2048)
    wo2 = np.clip(wo2, -240.0, 240.0).astype(fp8)
    wfT = np.transpose(np.asarray(Wf, f), (0, 2, 1)).reshape(L, 4, 128, D).astype(b16)
    p12 = np.stack(
        [np.broadcast_to(np.asarray(a, f)[:, None, :], (L, R, D))
         for a in (ln1_w, ln1_b, ln2_w, ln2_b, bo, bf)], axis=1)
    p12 = np.ascontiguousarray(p12)
    i128 = np.eye(128, dtype=b16)
    ones2 = np.ones((128, 32), f).astype(fp8)

    def to8(a):
        return np.clip(a, -240.0, 240.0).astype(fp8)

    in_maps = []
    for cc in range(NC):
        bs = slice(cc * BB, (cc + 1) * BB)
        pk = past_k[:, bs]                      # (L, BB, H, TP, Dh)
        pv = past_v[:, bs]
        # ktT[l, e, 64hf+d, 1024pr+t] = pk[l, e, 2pr+hf, t, d]
        kt = pk.reshape(L, BB, NPAIR, 2, TP, Dh)
        kt = np.transpose(kt, (0, 1, 3, 5, 2, 4))    # l, e, hf, d, pr, t
        kt = to8(np.ascontiguousarray(kt.reshape(L, BB, 128, NPAIR * TP)))
        # vF[l, e, p, 1024jp+512g+256i+128b+64hf+d] =
        #    pv[l, e, 4b+2g+hf, 512i+128jp+p, d]
        vf = pv.reshape(L, BB, 2, 2, 2, 2, NJ // 2, 128, Dh)
        #                      b  g  hf i  jp      p    d
        vf = np.transpose(vf, (0, 1, 7, 6, 3, 5, 2, 4, 8))
        #   -> l, e, p, jp, g, i, b, hf, d
        vf = to8(np.ascontiguousarray(vf.reshape(L, BB, 128, NPAIR * TP)))

        pm = np.asarray(pad_mask[bs])           # (BB, Tt) bool
        npad_e = (TP - pm[:, :TP].sum(axis=1)).astype(f)   # (BB,)

        npad4 = np.zeros((32, 2 * BB), f)
        for e in range(BB):
            npad4[0:12, 2 * e: 2 * e + 2] = npad_e[e]
        negnT = np.full((16, 256), NEG, f)
        for e in range(BB):
            for tn in range(TN):
                for g in range(2):
                    for m in range(12):
                        tq = m % 3
                        if tn <= tq and bool(pm[e, TP + tn]):
                            negnT[tn, 64 * e + 32 * g + m] = 0.0

        in_maps.append({
            "x0": np.ascontiguousarray(x[bs].reshape(R, D)),
            "ktT": kt, "vF": vf,
            "wq2": wq2, "wk2": wk2, "wv2": wv2,
            "wo2": wo2, "wfT": wfT, "p12": p12,
            "i128b": i128, "ones2f8": ones2,
            "negnT": negnT, "npad4": npad4,
        })
    return in_maps


_CACHE = {}


def kernel(**inputs):
    import os
    import sys
    for p in ("/opt/trn_rl_repo", "/opt/pypackages"):
        if p not in sys.path:
            sys.path.insert(0, p)
    os.environ.setdefault("JAX_PLATFORMS", "")
    from concourse.bass_utils import run_bass_kernel_spmd

    in_maps = _prep_inputs(**inputs)
    fast = all(np.allclose(np.asarray(inputs[k]), 1.0) for k in ("ln1_w", "ln2_w")) \
        and all(np.allclose(np.asarray(inputs[k]), 0.0)
                for k in ("ln1_b", "ln2_b", "bo", "bf"))
    key = f"nc_{fast}"
    if key not in _CACHE:
        _CACHE[key] = _build_bass(fast)
    nc = _CACHE[key]
    res = run_bass_kernel_spmd(nc, in_maps, core_ids=list(range(NC)))
    out = np.concatenate([r["xout"].reshape(BB, TN, D) for r in res.results], axis=0)
    return out.astype(np.float32)
